# revision 1
# baseline (speedup 1.0000x reference)
"""GQA attention (32 heads, 8 KV groups, rope, causal) on 8 TRN2 NeuronCores.

Sharding: tensor-parallel over KV groups — core g owns KV group g
(4 query heads + 1 kv head). Wq/Wk/Wv sharded column-wise, Wo row-wise;
each core produces a partial transposed output outT=[D,T], summed and
transposed on the host.

Per-core dataflow (T=2048 tokens, D=4096, head_dim=128):
  qT[dq,T] = wq.T @ xT   (accumulated over 32 k-tiles, psum chunked by 512 tokens)
  kT likewise; rope applied on the psum->sbuf copy; v transposed via PE.
  Per i-chunk I (512 queries), head pair: S^T[j,i] = k @ q^T -> +maskbias
    -> exp (ACT);  ctx^T[d,i] += v_j^T @ P^T,  rowsum[1,i] += ones^T @ P^T (PE)
    ctxT = psum_ctx * bcast(1/rowsum)   (ACT copy, gpsimd bcast, DVE recip/mul)
  outT[e,t] = sum_h wo_h^T-tile @ ctxT_h  (wo stationary reused over 4 t-chunks).

Matmuls run in float32r (TF32-like, 1 cycle/row) with fp32 PSUM accumulation.
"""
import math

import numpy as np

import concourse.bass as bass
import concourse.tile as tile
from concourse import bacc, mybir
from concourse.bass_utils import run_bass_kernel_spmd
from concourse.masks import make_identity

F32 = mybir.dt.float32
F32R = mybir.dt.float32r

T = 2048          # tokens
D = 4096          # model dim
HD = 128          # head dim
NH = 4            # heads per core
DQ = NH * HD      # 512 q dims per core
TC = 512          # token chunk (psum free dim)
NCH = T // TC     # 4 chunks
KT = D // 128     # 32 contraction tiles
JT = T // 128     # 16 key tiles
NET = D // 128    # 32 output-row tiles (of outT)
SCALE = 1.0 / math.sqrt(HD)
NCORES = 8


def build_nc():
    nc = bacc.Bacc("TRN2", target_bir_lowering=False, debug=False, num_devices=NCORES)
    xT = nc.dram_tensor("xT", [D, T], F32, kind="ExternalInput").ap()
    wq = nc.dram_tensor("wq", [D, DQ], F32, kind="ExternalInput").ap()
    wk = nc.dram_tensor("wk", [D, HD], F32, kind="ExternalInput").ap()
    wv = nc.dram_tensor("wv", [D, HD], F32, kind="ExternalInput").ap()
    wo = nc.dram_tensor("wo", [NET * 128, NH * 128], F32, kind="ExternalInput").ap()
    cosT = nc.dram_tensor("cosT", [HD, T], F32, kind="ExternalInput").ap()
    sinT = nc.dram_tensor("sinT", [HD, T], F32, kind="ExternalInput").ap()
    maskb = nc.dram_tensor("maskb", [128, 896], F32, kind="ExternalInput").ap()
    ones = nc.dram_tensor("ones", [128, 2], F32, kind="ExternalInput").ap()
    out = nc.dram_tensor("out", [D, T], F32, kind="ExternalOutput").ap()

    with tile.TileContext(nc) as tc:
        _body(tc, out, xT, wq, wk, wv, wo, cosT, sinT, maskb, ones)
    nc.compile()
    return nc


def _body(tc, out, xT, wq, wk, wv, wo, cosT, sinT, maskb, ones):
    nc = tc.nc
    from contextlib import ExitStack

    with ExitStack() as ctx:
        const_pool = ctx.enter_context(tc.tile_pool(name="const", bufs=1))
        w_pool = ctx.enter_context(tc.tile_pool(name="wp", bufs=1))
        x_pool = ctx.enter_context(tc.tile_pool(name="xp", bufs=3))
        qt_pool = ctx.enter_context(tc.tile_pool(name="qtp", bufs=4))
        kt_pool = ctx.enter_context(tc.tile_pool(name="ktp", bufs=4))
        v_pool = ctx.enter_context(tc.tile_pool(name="vp", bufs=16))
        vt_pool = ctx.enter_context(tc.tile_pool(name="vtp", bufs=1))
        pt_pool = ctx.enter_context(tc.tile_pool(name="ptp", bufs=4))
        cx_pool = ctx.enter_context(tc.tile_pool(name="cxp", bufs=16))
        rope_pool = ctx.enter_context(tc.tile_pool(name="ropep", bufs=2))
        rb_pool = ctx.enter_context(tc.tile_pool(name="rbp", bufs=1))
        rc_pool = ctx.enter_context(tc.tile_pool(name="rcp", bufs=4))
        wo_pool = ctx.enter_context(tc.tile_pool(name="wop", bufs=2))
        o_pool = ctx.enter_context(tc.tile_pool(name="op", bufs=2))
        ps_pool = ctx.enter_context(tc.tile_pool(name="ps", bufs=8, space="PSUM"))

        # ---- constants (scalar-engine DGE so the sync queue starts on x) ----
        mask_sb = const_pool.tile([128, 896], F32, tag="mask")
        ones_sb = const_pool.tile([128, 2], F32R, tag="ones")
        ident_sb = const_pool.tile([128, 128], F32, tag="ident")
        nc.scalar.dma_start(mask_sb[:], maskb[:, :])
        nc.scalar.dma_start(ones_sb[:], ones[:, :].bitcast(F32R))
        make_identity(nc, ident_sb[:])
        cs_pool = ctx.enter_context(tc.tile_pool(name="csp", bufs=2))

        # ---- resident weights (f32r); loaded inside chunk-0 k-loop so the
        # sync queue serves the first matmuls' inputs immediately ----
        wq_sb = w_pool.tile([128, KT * DQ], F32R, tag="wq")
        wk_sb = w_pool.tile([128, KT * HD], F32R, tag="wk")
        wv_sb = w_pool.tile([128, KT * HD], F32R, tag="wv")

        kt_tiles = []      # kT chunk tiles [128, TC] (d x tokens), f32r
        v_tiles = []       # v j-tiles [128, 128] (tokens x d), f32r
        cx_tiles = {}      # (h, chunk) -> ctxT tile [128, TC], f32r

        for c in range(NCH):
            # ================= projections for token chunk c =================
            ps_q = [ps_pool.tile([128, TC], F32, tag="ps", name=f"psq{h}_{c}")
                    for h in range(NH)]
            ps_k = ps_pool.tile([128, TC], F32, tag="ps", name=f"psk_{c}")
            ps_v = ps_pool.tile([128, TC], F32, tag="ps", name=f"psv_{c}")
            for k in range(KT):
                if c == 0:
                    nc.sync.dma_start(
                        wq_sb[:, k * DQ:(k + 1) * DQ],
                        wq[k * 128:(k + 1) * 128, :].bitcast(F32R),
                    )
                    nc.sync.dma_start(
                        wk_sb[:, k * HD:(k + 1) * HD],
                        wk[k * 128:(k + 1) * 128, :].bitcast(F32R),
                    )
                    nc.sync.dma_start(
                        wv_sb[:, k * HD:(k + 1) * HD],
                        wv[k * 128:(k + 1) * 128, :].bitcast(F32R),
                    )
                xt = x_pool.tile([128, TC], F32R, tag="x", name=f"x_{c}_{k}")
                nc.sync.dma_start(
                    xt[:], xT[k * 128:(k + 1) * 128, c * TC:(c + 1) * TC].bitcast(F32R)
                )
                first, last = k == 0, k == KT - 1
                for h in range(NH):
                    nc.tensor.matmul(
                        ps_q[h][:],
                        wq_sb[:, k * DQ + h * HD:k * DQ + (h + 1) * HD],
                        xt[:],
                        start=first, stop=last,
                    )
                nc.tensor.matmul(
                    ps_k[:], wk_sb[:, k * HD:(k + 1) * HD], xt[:],
                    start=first, stop=last,
                )
                nc.tensor.matmul(
                    ps_v[:], wv_sb[:, k * HD:(k + 1) * HD], xt[:],
                    start=first, stop=last,
                )

            cs_t = cs_pool.tile([HD, TC], F32, tag="cos", name=f"cos_{c}")
            sn_t = cs_pool.tile([HD, TC], F32, tag="sin", name=f"sin_{c}")
            nc.scalar.dma_start(cs_t[:], cosT[:, c * TC:(c + 1) * TC])
            nc.scalar.dma_start(sn_t[:], sinT[:, c * TC:(c + 1) * TC])
            cs = cs_t[:, :]
            sn = sn_t[:, :]

            def rope(ps, dst_pool, tag, nm):
                t1 = rope_pool.tile([128, TC], F32, tag="t1", name=f"r1{nm}")
                t2 = rope_pool.tile([128, TC], F32, tag="t2", name=f"r2{nm}")
                nc.vector.tensor_mul(t2[0:64, :], ps[64:128, :], sn[0:64, :])
                nc.vector.tensor_mul(t2[64:128, :], ps[0:64, :], sn[64:128, :])
                nc.vector.tensor_mul(t1[:], ps[:], cs)
                d = dst_pool.tile([128, TC], F32R, tag=tag, name=nm)
                nc.vector.tensor_add(d[:], t1[:], t2[:])
                return d

            # k first: attention needs kt before S matmuls
            kt = rope(ps_k, kt_pool, "kt", f"kt_{c}")
            kt_tiles.append(kt)

            # v: psum -> sbuf, then PE-transpose each [128,128] to tokens-major
            vt = vt_pool.tile([128, TC], F32, tag="vt", name=f"vt_{c}")
            nc.scalar.copy(vt[:], ps_v[:])
            for jj in range(TC // 128):
                ps_t = ps_pool.tile([128, 128], F32, tag="ps",
                                    name=f"pst_{c}_{jj}")
                nc.tensor.transpose(ps_t[:], vt[:, jj * 128:(jj + 1) * 128],
                                    ident_sb[:])
                vsb = v_pool.tile([128, 128], F32R, tag="v", name=f"v_{c}_{jj}")
                nc.vector.tensor_copy(vsb[:], ps_t[:])
                v_tiles.append(vsb)

            q_chunk = [rope(ps_q[h], qt_pool, "qt", f"qt_{c}_{h}")
                       for h in range(NH)]

            # ========== attention for i-chunk I = c, two heads at a time =====
            I = c
            nj = 4 * I + 4
            ctx_un = {}
            sum_rows = {}
            for hp in range(NH // 2):
                hs = [2 * hp, 2 * hp + 1]
                ps_ctx = {h: ps_pool.tile([128, TC], F32, tag="ps",
                                          name=f"psctx_{I}_{h}") for h in hs}
                ps_sum = {h: ps_pool.tile([2, TC], F32, tag="ps",
                                          name=f"pssum_{I}_{h}") for h in hs}
                for J in range(nj):
                    pts = {}
                    for h in hs:  # kT_J stationary shared across the pair
                        ps_s = ps_pool.tile([128, TC], F32, tag="ps",
                                            name=f"pss_{I}_{h}_{J}")
                        nc.tensor.matmul(
                            ps_s[:],
                            kt_tiles[J // 4][:, (J % 4) * 128:(J % 4 + 1) * 128],
                            q_chunk[h][:],
                            start=True, stop=True,
                        )
                        if J >= 4 * I:  # diagonal tile: additive causal mask
                            q = J - 4 * I
                            off = (3 - q) * 128
                            nc.vector.tensor_add(
                                ps_s[:], ps_s[:], mask_sb[:, off:off + TC]
                            )
                        pt = pt_pool.tile([128, TC], F32R, tag="pt",
                                          name=f"pt_{I}_{h}_{J}")
                        nc.scalar.activation(
                            pt[:], ps_s[:], mybir.ActivationFunctionType.Exp,
                            scale=SCALE,
                        )
                        pts[h] = pt
                    first, last = J == 0, J == nj - 1
                    for h in hs:  # v_J stationary shared across the pair
                        nc.tensor.matmul(ps_ctx[h][:], v_tiles[J][:], pts[h][:],
                                         start=first, stop=last)
                    for h in hs:  # ones stationary (trivial ldweights)
                        nc.tensor.matmul(ps_sum[h][:], ones_sb[:], pts[h][:],
                                         start=first, stop=last)

                for h in hs:
                    # free psum fast: tiny ACT copy of the sums row, and a
                    # DVE/ACT copy of ctx (alternating pairs to balance load)
                    srow = rc_pool.tile([1, TC], F32, tag="recip",
                                        name=f"rc_{I}_{h}")
                    nc.scalar.copy(srow[:], ps_sum[h][0:1, :])
                    sum_rows[h] = srow
                    cxt = cx_pool.tile([128, TC], F32R, tag="cx",
                                       name=f"cx_{I}_{h}")
                    if hp == 0:
                        nc.vector.tensor_copy(cxt[:], ps_ctx[h][:])
                    else:
                        nc.scalar.copy(cxt[:], ps_ctx[h][:])
                    ctx_un[h] = cxt

            # per-head broadcast + reciprocal + scale (out of the psum path)
            for h in range(NH):
                rb = rb_pool.tile([128, TC], F32, tag="rb", name=f"rb_{I}_{h}")
                nc.gpsimd.partition_broadcast(rb[:], sum_rows[h][:])
                nc.vector.reciprocal(rb[:], rb[:])
                cxt = ctx_un[h]
                nc.vector.tensor_mul(cxt[:], cxt[:], rb[:])
                cx_tiles[(h, I)] = cxt

        # ======= output stage: outT[e,t], wo-tile stationary reused 4x =======
        for Et in range(NET):
            woe = wo_pool.tile([128, NH * 128], F32R, tag="wo", name=f"wo_{Et}")
            nc.sync.dma_start(
                woe[:], wo[Et * 128:(Et + 1) * 128, :].bitcast(F32R)
            )
            ps_o = [ps_pool.tile([128, TC], F32, tag="ps", name=f"pso_{Et}_{tc_}")
                    for tc_ in range(NCH)]
            for h in range(NH):
                for tc_ in range(NCH):
                    nc.tensor.matmul(
                        ps_o[tc_][:],
                        woe[:, h * 128:(h + 1) * 128],
                        cx_tiles[(h, tc_)][:],
                        start=h == 0, stop=h == NH - 1,
                    )
            for tc_ in range(NCH):
                ot = o_pool.tile([128, TC], F32, tag="o", name=f"o_{Et}_{tc_}")
                if tc_ % 2 == 0:
                    nc.vector.tensor_copy(ot[:], ps_o[tc_][:])
                else:
                    nc.scalar.copy(ot[:], ps_o[tc_][:])
                nc.sync.dma_start(
                    out[Et * 128:(Et + 1) * 128, tc_ * TC:(tc_ + 1) * TC], ot[:]
                )


# ---------------------------------------------------------------------------
# host side
# ---------------------------------------------------------------------------
_NC_CACHE = None


def _get_nc():
    global _NC_CACHE
    if _NC_CACHE is None:
        _NC_CACHE = build_nc()
    return _NC_CACHE


def make_in_maps(x, Wq, Wk, Wv, Wo, cos, sin):
    x = np.asarray(x, dtype=np.float32)
    xT = np.ascontiguousarray(x.reshape(T, D).T)
    cosT = np.ascontiguousarray(np.asarray(cos, np.float32)[:T].T)
    sin_t = np.asarray(sin, np.float32)[:T]          # [T, 128]
    sinT = sin_t.T.copy()                            # [128, T]
    sinT[:64] *= -1.0                                # fold rotate-half sign
    sinT = np.ascontiguousarray(sinT)

    # sliding additive causal mask: tile q reads cols (3-q)*128 : (3-q)*128+512
    # of big[r, cc] = 0 if cc >= 384 + r else -1e30
    r = np.arange(128)[:, None]
    cc = np.arange(896)[None, :]
    m = np.where(cc >= 384 + r, 0.0, -1.0e30).astype(np.float32)
    ones = np.ones((128, 2), np.float32)

    Wq = np.asarray(Wq, np.float32)
    Wk = np.asarray(Wk, np.float32)
    Wv = np.asarray(Wv, np.float32)
    Wo = np.asarray(Wo, np.float32)
    # per core: wo rows [g*DQ:(g+1)*DQ] shuffled to [Et, dh, (h, e)] so each
    # Et-tile is one contiguous [128, NH*128] DMA with 2KB runs
    woP = np.empty((NCORES, NET * 128, NH * 128), np.float32)
    for g in range(NCORES):
        w = Wo[g * DQ:(g + 1) * DQ, :]                    # [512, 4096]
        w4 = w.reshape(NH, HD, NET, 128).transpose(2, 1, 0, 3)  # [Et, dh, h, e]
        woP[g] = w4.reshape(NET * 128, NH * 128)
    in_maps = []
    for g in range(NCORES):
        in_maps.append({
            "xT": xT,
            "wq": np.ascontiguousarray(Wq[:, g * DQ:(g + 1) * DQ]),
            "wk": np.ascontiguousarray(Wk[:, g * HD:(g + 1) * HD]),
            "wv": np.ascontiguousarray(Wv[:, g * HD:(g + 1) * HD]),
            "wo": woP[g],
            "cosT": cosT,
            "sinT": sinT,
            "maskb": m,
            "ones": ones,
        })
    return in_maps


def kernel(x, Wq, Wk, Wv, Wo, cos, sin):
    nc = _get_nc()
    in_maps = make_in_maps(x, Wq, Wk, Wv, Wo, cos, sin)
    res = run_bass_kernel_spmd(nc, in_maps, core_ids=list(range(NCORES)))
    acc = np.zeros((D, T), np.float32)
    for c in range(NCORES):
        acc += res.results[c]["out"]
    return np.ascontiguousarray(acc.T).reshape(1, T, D)



# revision 25
# speedup vs baseline: 1.4477x; 1.4477x over previous
"""GQA attention (32 heads, 8 KV groups, rope, causal) on 8 TRN2 NeuronCores.

Sharding: tensor-parallel over KV groups — core g owns KV group g
(4 query heads + 1 kv head). Wq/Wk/Wv sharded column-wise, Wo row-wise;
each core produces a partial transposed output outT=[D,T] in bf16,
summed in fp32 and transposed on the host.

Per-core dataflow (T=2048 tokens, D=4096, head_dim=128), all matmuls
bf16 with fp32 PSUM:
  proj:  pair-psums q01/q23/kv accumulate over 32 k-tiles; psum halves
         are ACT-evacuated to bf16 and rope runs as 4 bf16 DVE ops.
         v is PE-transposed (4 blocks into one packed psum) to tokens-major.
  attn (chunk I = 512 queries, 2 heads at a time, software-pipelined):
         S-pair[j, i|i'] = kt_J @ (q_h0|q_h1)  -> masked adds (DVE, width-
         restricted) -> one exp (ACT) -> pt pair (bf16)
         ctx-pair += v_J^T @ pt halves;  rowsums via ones-matmuls into one
         psum bank at partition 0 / 32 (col-group pair).
         Normalization: batched DVE reciprocal of the 4 rowsum rows per
         chunk, gpsimd partition-broadcast, DVE multiply.
  out:   resident wo (bf16) stationary tiles; paired [128,1024] psum,
         paired copies and 2KB-line DMAs.
DMA queues: x + out on sync HWDGE, weights (wq/wk/wv/wo) on gpsimd SWDGE,
constants on scalar HWDGE.
"""
import math

import ml_dtypes
import numpy as np

import concourse.bass as bass
import concourse.tile as tile
from concourse import bacc, mybir
from concourse.bass_utils import run_bass_kernel_spmd
from concourse.masks import make_identity

F32 = mybir.dt.float32
BF16 = mybir.dt.bfloat16
NPBF16 = ml_dtypes.bfloat16

T = 2048          # tokens
D = 4096          # model dim
HD = 128          # head dim
NH = 4            # heads per core
DQ = NH * HD      # 512 q dims per core
TC = 512          # token chunk (psum free dim)
NCH = T // TC     # 4 chunks
KT = D // 128     # 32 contraction tiles
JT = T // 128     # 16 key tiles
NET = D // 128    # 32 output-row tiles (of outT)
SCALE = 1.0 / math.sqrt(HD)
NCORES = 8
EXPF = mybir.ActivationFunctionType.Exp


def build_nc():
    nc = bacc.Bacc("TRN2", target_bir_lowering=False, debug=False, num_devices=NCORES)
    xT = nc.dram_tensor("xT", [D, T], BF16, kind="ExternalInput").ap()
    wq = nc.dram_tensor("wq", [128, KT * DQ], BF16, kind="ExternalInput").ap()
    wk = nc.dram_tensor("wk", [128, KT * HD], BF16, kind="ExternalInput").ap()
    wv = nc.dram_tensor("wv", [128, KT * HD], BF16, kind="ExternalInput").ap()
    wo = nc.dram_tensor("wo", [128, NET * DQ], BF16, kind="ExternalInput").ap()
    cosT = nc.dram_tensor("cosT", [HD, T], BF16, kind="ExternalInput").ap()
    sinT = nc.dram_tensor("sinT", [HD, T], BF16, kind="ExternalInput").ap()
    maskb = nc.dram_tensor("maskb", [128, 896], F32, kind="ExternalInput").ap()
    ones = nc.dram_tensor("ones", [128, 2], BF16, kind="ExternalInput").ap()
    out = nc.dram_tensor("out", [D, T], BF16, kind="ExternalOutput").ap()

    with tile.TileContext(nc) as tc:
        _body(tc, out, xT, wq, wk, wv, wo, cosT, sinT, maskb, ones)
    nc.compile()
    return nc


def _body(tc, out, xT, wq, wk, wv, wo, cosT, sinT, maskb, ones):
    nc = tc.nc
    from contextlib import ExitStack

    with ExitStack() as ctx:
        const_pool = ctx.enter_context(tc.tile_pool(name="const", bufs=1))
        w_pool = ctx.enter_context(tc.tile_pool(name="wp", bufs=1))
        x_pool = ctx.enter_context(tc.tile_pool(name="xp", bufs=4))
        qt_pool = ctx.enter_context(tc.tile_pool(name="qtp", bufs=4))
        kt_pool = ctx.enter_context(tc.tile_pool(name="ktp", bufs=4))
        v4_pool = ctx.enter_context(tc.tile_pool(name="v4p", bufs=4))
        vt_pool = ctx.enter_context(tc.tile_pool(name="vtp", bufs=1))
        pt_pool = ctx.enter_context(tc.tile_pool(name="ptp", bufs=3))
        cx_pool = ctx.enter_context(tc.tile_pool(name="cxp", bufs=16))
        rope_pool = ctx.enter_context(tc.tile_pool(name="ropep", bufs=2))
        rb_pool = ctx.enter_context(tc.tile_pool(name="rbp", bufs=2))
        sr_pool = ctx.enter_context(tc.tile_pool(name="srp", bufs=2))
        o_pool = ctx.enter_context(tc.tile_pool(name="op", bufs=4))
        cs_pool = ctx.enter_context(tc.tile_pool(name="csp", bufs=2))
        ps_pool = ctx.enter_context(tc.tile_pool(name="ps", bufs=3, space="PSUM"))

        # ---- constants (scalar HWDGE queue) ----
        mask_sb = const_pool.tile([128, 896], F32, tag="mask")
        ones_sb = const_pool.tile([128, 2], BF16, tag="ones")
        ident_sb = const_pool.tile([128, 128], BF16, tag="ident")
        nc.scalar.dma_start(mask_sb[:], maskb[:, :])
        nc.scalar.dma_start(ones_sb[:], ones[:, :])
        make_identity(nc, ident_sb[:])

        # ---- resident weights (bf16, partition-major host layout) on the
        # gpsimd SWDGE queue so they never contend with x on sync ----
        wq_sb = w_pool.tile([128, KT * DQ], BF16, tag="wq")
        wk_sb = w_pool.tile([128, KT * HD], BF16, tag="wk")
        wv_sb = w_pool.tile([128, KT * HD], BF16, tag="wv")
        wo_sb = w_pool.tile([128, NET * DQ], BF16, tag="wo")
        for g in range(8):  # 4 k-tiles per transfer, k/v interleaved so the
            nc.gpsimd.dma_start(  # chunk-0 k-loop's deps arrive in k order
                wq_sb[:, g * 4 * DQ:(g + 1) * 4 * DQ],
                wq[:, g * 4 * DQ:(g + 1) * 4 * DQ],
            )
            nc.gpsimd.dma_start(
                wk_sb[:, g * 4 * HD:(g + 1) * 4 * HD],
                wk[:, g * 4 * HD:(g + 1) * 4 * HD],
            )
            nc.gpsimd.dma_start(
                wv_sb[:, g * 4 * HD:(g + 1) * 4 * HD],
                wv[:, g * 4 * HD:(g + 1) * 4 * HD],
            )

        kt_tiles = []      # kT chunk tiles [128, TC] (d x tokens), bf16
        v4_tiles = []      # packed vT tiles [128, TC] (tokens x d), bf16
        cx_tiles = {}      # (h, chunk) -> ctxT tile [128, TC], bf16

        for c in range(NCH):
            # ================= projections for token chunk c =================
            ps_q01 = ps_pool.tile([128, 2 * TC], F32, tag="pair",
                                  name=f"psq01_{c}")
            ps_q23 = ps_pool.tile([128, 2 * TC], F32, tag="pair",
                                  name=f"psq23_{c}")
            ps_kv = ps_pool.tile([128, 2 * TC], F32, tag="pair",
                                 name=f"pskv_{c}")
            for k in range(KT):
                xt = x_pool.tile([128, TC], BF16, tag="x", name=f"x_{c}_{k}")
                nc.sync.dma_start(
                    xt[:], xT[k * 128:(k + 1) * 128, c * TC:(c + 1) * TC]
                )
                first, last = k == 0, k == KT - 1
                for h in range(NH):
                    dst = ps_q01 if h < 2 else ps_q23
                    nc.tensor.matmul(
                        dst[:, (h % 2) * TC:(h % 2 + 1) * TC],
                        wq_sb[:, k * DQ + h * HD:k * DQ + (h + 1) * HD],
                        xt[:],
                        start=first, stop=last,
                    )
                nc.tensor.matmul(
                    ps_kv[:, 0:TC], wk_sb[:, k * HD:(k + 1) * HD], xt[:],
                    start=first, stop=last,
                )
                nc.tensor.matmul(
                    ps_kv[:, TC:2 * TC], wv_sb[:, k * HD:(k + 1) * HD], xt[:],
                    start=first, stop=last,
                )

            if c == 0:  # wo after the chunk-0 weights on the same queue
                for g in range(4):
                    nc.gpsimd.dma_start(
                        wo_sb[:, g * 8 * DQ:(g + 1) * 8 * DQ],
                        wo[:, g * 8 * DQ:(g + 1) * 8 * DQ],
                    )

            cs_t = cs_pool.tile([HD, TC], BF16, tag="cos", name=f"cos_{c}")
            sn_t = cs_pool.tile([HD, TC], BF16, tag="sin", name=f"sin_{c}")
            nc.scalar.dma_start(cs_t[:], cosT[:, c * TC:(c + 1) * TC])
            nc.scalar.dma_start(sn_t[:], sinT[:, c * TC:(c + 1) * TC])
            cs = cs_t[:, :]
            sn = sn_t[:, :]

            def rope(ps_half, dst_pool, tag, nm):
                # psum-direct muls (crossed reads must come from PSUM — the
                # verifier requires SBUF operands to share start partitions),
                # bf16 outputs so the final add runs in the 2x DVE mode
                t1 = rope_pool.tile([128, TC], BF16, tag="t1", name=f"r1{nm}")
                t2 = rope_pool.tile([128, TC], BF16, tag="t2", name=f"r2{nm}")
                nc.vector.tensor_mul(t2[0:64, :], ps_half[64:128, :],
                                     sn[0:64, :])
                nc.vector.tensor_mul(t2[64:128, :], ps_half[0:64, :],
                                     sn[64:128, :])
                nc.vector.tensor_mul(t1[:], ps_half, cs)
                d = dst_pool.tile([128, TC], BF16, tag=tag, name=nm)
                nc.vector.tensor_add(d[:], t1[:], t2[:])
                return d

            # k first (chunk-0 attention needs it immediately), then q0/q1
            kt = rope(ps_kv[:, 0:TC], kt_pool, "kt", f"kt_{c}")
            kt_tiles.append(kt)
            q_chunk = [None] * NH
            q_chunk[0] = rope(ps_q01[:, 0:TC], qt_pool, "qt", f"qt_{c}_0")
            q_chunk[1] = rope(ps_q01[:, TC:2 * TC], qt_pool, "qt", f"qt_{c}_1")

            # v: ACT copy to bf16, 4 PE transposes into one packed psum tile,
            # one DVE cast out
            vt = vt_pool.tile([128, TC], BF16, tag="vt", name=f"vt_{c}")
            nc.scalar.copy(vt[:], ps_kv[:, TC:2 * TC])
            ps_t = ps_pool.tile([128, TC], BF16, tag="pair",
                                name=f"pst_{c}")
            for jj in range(TC // 128):
                nc.tensor.transpose(ps_t[:, jj * 128:(jj + 1) * 128],
                                    vt[:, jj * 128:(jj + 1) * 128],
                                    ident_sb[:])
            v4 = v4_pool.tile([128, TC], BF16, tag="v4", name=f"v4_{c}")
            nc.vector.tensor_copy(v4[:], ps_t[:])
            v4_tiles.append(v4)

            q_chunk[2] = rope(ps_q23[:, 0:TC], qt_pool, "qt", f"qt_{c}_2")
            q_chunk[3] = rope(ps_q23[:, TC:2 * TC], qt_pool, "qt", f"qt_{c}_3")

            # ========== attention for i-chunk I = c, two heads at a time =====
            # Each head's rowsum accumulation group gets its OWN psum bank
            # (start=True clears has_written state per bank; sharing a bank
            # between groups accumulates onto stale data).
            I = c
            nj = 4 * I + 4
            ctx_un = {}
            srh = {}
            for hp in range(NH // 2):
                h0, h1 = 2 * hp, 2 * hp + 1
                ps_ctx = ps_pool.tile([128, 2 * TC], F32, tag="pair",
                                      name=f"psctx_{I}_{hp}")
                # per-head rowsum groups in their OWN banks (start=True
                # clears has_written state; groups must not share a bank)
                ps_s0 = ps_pool.tile([2, TC], F32, tag="one", bufs=2,
                                     name=f"pssum_{I}_{h0}")
                ps_s1 = ps_pool.tile([2, TC], F32, tag="one", bufs=2,
                                     name=f"pssum_{I}_{h1}")
                pts = {}

                def ctx_ones(J):
                    first, last = J == 0, J == nj - 1
                    pt2 = pts.pop(J)
                    vst = v4_tiles[J // 4][:, (J % 4) * 128:(J % 4 + 1) * 128]
                    nc.tensor.matmul(ps_ctx[:, 0:TC], vst, pt2[:, 0:TC],
                                     start=first, stop=last)
                    nc.tensor.matmul(ps_ctx[:, TC:2 * TC], vst,
                                     pt2[:, TC:2 * TC],
                                     start=first, stop=last)
                    nc.tensor.matmul(ps_s0[:], ones_sb[:], pt2[:, 0:TC],
                                     start=first, stop=last)
                    nc.tensor.matmul(ps_s1[:], ones_sb[:], pt2[:, TC:2 * TC],
                                     start=first, stop=last)

                for J in range(nj):
                    s2 = ps_pool.tile([128, 2 * TC], F32, tag="pair",
                                      name=f"pss_{I}_{hp}_{J}")
                    kst = kt_tiles[J // 4][:, (J % 4) * 128:(J % 4 + 1) * 128]
                    nc.tensor.matmul(s2[:, 0:TC], kst, q_chunk[h0][:],
                                     start=True, stop=True)
                    nc.tensor.matmul(s2[:, TC:2 * TC], kst, q_chunk[h1][:],
                                     start=True, stop=True)
                    if J >= 4 * I:  # diagonal: additive causal mask, only
                        q = J - 4 * I          # the first (q+1)*128 cols vary
                        w = (q + 1) * 128
                        off = (3 - q) * 128
                        nc.vector.tensor_add(
                            s2[:, 0:w], s2[:, 0:w], mask_sb[:, off:off + w]
                        )
                        nc.vector.tensor_add(
                            s2[:, TC:TC + w], s2[:, TC:TC + w],
                            mask_sb[:, off:off + w]
                        )
                    pt2 = pt_pool.tile([128, 2 * TC], BF16, tag="pt",
                                       name=f"pt_{I}_{hp}_{J}")
                    nc.scalar.activation(pt2[:], s2[:], EXPF, scale=SCALE)
                    pts[J] = pt2
                    if J >= 1:
                        ctx_ones(J - 1)
                ctx_ones(nj - 1)

                # evacuate: rowsum copies (frees the sum banks for the next
                # head pair) + immediate [1,TC] reciprocal, ctx halves on
                # DVE/ACT
                for h, ps_s in ((h0, ps_s0), (h1, ps_s1)):
                    sl = sr_pool.tile([1, TC], F32, tag="sl", bufs=6,
                                      name=f"sl_{I}_{h}")
                    nc.scalar.copy(sl[:], ps_s[0:1, :])
                    nc.vector.reciprocal(sl[:], sl[:])
                    srh[h] = sl
                cx0 = cx_pool.tile([128, TC], BF16, tag="cx",
                                   name=f"cx_{I}_{h0}")
                nc.vector.tensor_copy(cx0[:], ps_ctx[:, 0:TC])
                ctx_un[h0] = cx0
                cx1 = cx_pool.tile([128, TC], BF16, tag="cx",
                                   name=f"cx_{I}_{h1}")
                nc.scalar.copy(cx1[:], ps_ctx[:, TC:2 * TC])
                ctx_un[h1] = cx1

            # per-head broadcast + scale — off the psum path
            for h in range(NH):
                rb = rb_pool.tile([128, TC], F32, tag="rb", name=f"rb_{I}_{h}")
                nc.gpsimd.partition_broadcast(rb[:], srh[h][:])
                cxt = ctx_un[h]
                nc.vector.tensor_mul(cxt[:], cxt[:], rb[:])
                cx_tiles[(h, I)] = cxt

        # ======= output stage: outT[e,t], resident wo stationary tiles =======
        for Et in range(NET):
            ps_o = [ps_pool.tile([128, 2 * TC], F32, tag="pair",
                                 name=f"pso_{Et}_{p}") for p in range(2)]
            for h in range(NH):
                wst = wo_sb[:, Et * DQ + h * HD:Et * DQ + (h + 1) * HD]
                for tc_ in range(NCH):
                    nc.tensor.matmul(
                        ps_o[tc_ // 2][:, (tc_ % 2) * TC:(tc_ % 2 + 1) * TC],
                        wst,
                        cx_tiles[(h, tc_)][:],
                        start=h == 0, stop=h == NH - 1,
                    )
            for p in range(2):
                ot = o_pool.tile([128, 2 * TC], BF16, tag="o",
                                 name=f"o_{Et}_{p}")
                if p == 0:
                    nc.vector.tensor_copy(ot[:], ps_o[p][:])
                else:
                    nc.scalar.copy(ot[:], ps_o[p][:])
                nc.sync.dma_start(
                    out[Et * 128:(Et + 1) * 128,
                        p * 2 * TC:(p + 1) * 2 * TC],
                    ot[:],
                )


# ---------------------------------------------------------------------------
# host side
# ---------------------------------------------------------------------------
_NC_CACHE = None


def _get_nc():
    global _NC_CACHE
    if _NC_CACHE is None:
        _NC_CACHE = build_nc()
    return _NC_CACHE


def _pmajor(w, kt, width):
    """[kt*128, width] -> partition-major [128, kt*width] bf16."""
    return np.ascontiguousarray(
        w.reshape(kt, 128, width).transpose(1, 0, 2).reshape(128, kt * width)
    )


def make_in_maps(x, Wq, Wk, Wv, Wo, cos, sin):
    x = np.asarray(x, dtype=np.float32)
    xT = np.ascontiguousarray(x.reshape(T, D).T.astype(NPBF16))
    cosT = np.ascontiguousarray(
        np.asarray(cos, np.float32)[:T].T.astype(NPBF16))
    sin_t = np.asarray(sin, np.float32)[:T]          # [T, 128]
    sinT = sin_t.T.copy()                            # [128, T]
    sinT[:64] *= -1.0                                # fold rotate-half sign
    sinT = np.ascontiguousarray(sinT.astype(NPBF16))

    # sliding additive causal mask: diagonal tile q reads cols
    # (3-q)*128 : (3-q)*128 + (q+1)*128 of big[r, cc] = 0 if cc >= 384 + r
    r = np.arange(128)[:, None]
    cc = np.arange(896)[None, :]
    m = np.where(cc >= 384 + r, 0.0, -1.0e30).astype(np.float32)
    ones = np.ones((128, 2), NPBF16)

    Wq = np.asarray(Wq, np.float32).astype(NPBF16)
    Wk = np.asarray(Wk, np.float32).astype(NPBF16)
    Wv = np.asarray(Wv, np.float32).astype(NPBF16)
    Wo = np.asarray(Wo, np.float32).astype(NPBF16)
    in_maps = []
    for g in range(NCORES):
        # wo rows [g*DQ:(g+1)*DQ] shuffled to [dh, (Et, h, e)] so Et-tiles are
        # resident stationary slices of one partition-major tensor
        w = Wo[g * DQ:(g + 1) * DQ, :]                          # [512, 4096]
        w4 = w.reshape(NH, HD, NET, 128).transpose(1, 2, 0, 3)  # [dh,Et,h,e]
        woP = np.ascontiguousarray(w4.reshape(128, NET * DQ))
        in_maps.append({
            "xT": xT,
            "wq": _pmajor(Wq[:, g * DQ:(g + 1) * DQ], KT, DQ),
            "wk": _pmajor(Wk[:, g * HD:(g + 1) * HD], KT, HD),
            "wv": _pmajor(Wv[:, g * HD:(g + 1) * HD], KT, HD),
            "wo": woP,
            "cosT": cosT,
            "sinT": sinT,
            "maskb": m,
            "ones": ones,
        })
    return in_maps


def kernel(x, Wq, Wk, Wv, Wo, cos, sin):
    nc = _get_nc()
    in_maps = make_in_maps(x, Wq, Wk, Wv, Wo, cos, sin)
    res = run_bass_kernel_spmd(nc, in_maps, core_ids=list(range(NCORES)))
    acc = np.zeros((D, T), np.float32)
    for c in range(NCORES):
        acc += res.results[c]["out"].astype(np.float32)
    return np.ascontiguousarray(acc.T).reshape(1, T, D)


# revision 29
# speedup vs baseline: 1.4744x; 1.0184x over previous
"""GQA attention (32 heads, 8 KV groups, rope, causal) on 8 TRN2 NeuronCores.

Sharding: tensor-parallel over KV groups — core g owns KV group g
(4 query heads + 1 kv head). Wq/Wk/Wv sharded column-wise, Wo row-wise;
each core produces a partial transposed output outT=[D,T] in bf16,
summed in fp32 and transposed on the host.

Per-core dataflow (T=2048 tokens, D=4096, head_dim=128), all matmuls
bf16 with fp32 PSUM:
  proj:  pair-psums q01/q23/kv accumulate over 32 k-tiles; psum halves
         are ACT-evacuated to bf16 and rope runs as 4 bf16 DVE ops.
         v is PE-transposed (4 blocks into one packed psum) to tokens-major.
  attn (chunk I = 512 queries, 2 heads at a time, software-pipelined):
         S-pair[j, i|i'] = kt_J @ (q_h0|q_h1)  -> masked adds (DVE, width-
         restricted) -> one exp (ACT) -> pt pair (bf16)
         ctx-pair += v_J^T @ pt halves;  rowsums via ones-matmuls into one
         psum bank at partition 0 / 32 (col-group pair).
         Normalization: batched DVE reciprocal of the 4 rowsum rows per
         chunk, gpsimd partition-broadcast, DVE multiply.
  out:   resident wo (bf16) stationary tiles; paired [128,1024] psum,
         paired copies and 2KB-line DMAs.
DMA queues: x + out on sync HWDGE, weights (wq/wk/wv/wo) on gpsimd SWDGE,
constants on scalar HWDGE.
"""
import math

import ml_dtypes
import numpy as np

import concourse.bass as bass
import concourse.tile as tile
from concourse import bacc, mybir
from concourse.bass_utils import run_bass_kernel_spmd
from concourse.masks import make_identity

F32 = mybir.dt.float32
BF16 = mybir.dt.bfloat16
NPBF16 = ml_dtypes.bfloat16

T = 2048          # tokens
D = 4096          # model dim
HD = 128          # head dim
NH = 4            # heads per core
DQ = NH * HD      # 512 q dims per core
TC = 512          # token chunk (psum free dim)
NCH = T // TC     # 4 chunks
KT = D // 128     # 32 contraction tiles
JT = T // 128     # 16 key tiles
NET = D // 128    # 32 output-row tiles (of outT)
SCALE = 1.0 / math.sqrt(HD)
NCORES = 8
EXPF = mybir.ActivationFunctionType.Exp


def build_nc():
    nc = bacc.Bacc("TRN2", target_bir_lowering=False, debug=False, num_devices=NCORES)
    xT = nc.dram_tensor("xT", [D, T], BF16, kind="ExternalInput").ap()
    wq = nc.dram_tensor("wq", [128, KT * DQ], BF16, kind="ExternalInput").ap()
    wk = nc.dram_tensor("wk", [128, KT * HD], BF16, kind="ExternalInput").ap()
    wv = nc.dram_tensor("wv", [128, KT * HD], BF16, kind="ExternalInput").ap()
    wo = nc.dram_tensor("wo", [128, NET * DQ], BF16, kind="ExternalInput").ap()
    cosT = nc.dram_tensor("cosT", [HD, T], BF16, kind="ExternalInput").ap()
    sinT = nc.dram_tensor("sinT", [HD, T], BF16, kind="ExternalInput").ap()
    maskb = nc.dram_tensor("maskb", [128, 896], F32, kind="ExternalInput").ap()
    ones = nc.dram_tensor("ones", [128, 2], BF16, kind="ExternalInput").ap()
    out = nc.dram_tensor("out", [D, T], BF16, kind="ExternalOutput").ap()

    with tile.TileContext(nc) as tc:
        _body(tc, out, xT, wq, wk, wv, wo, cosT, sinT, maskb, ones)
    nc.compile()
    return nc


def _body(tc, out, xT, wq, wk, wv, wo, cosT, sinT, maskb, ones):
    nc = tc.nc
    from contextlib import ExitStack

    with ExitStack() as ctx:
        const_pool = ctx.enter_context(tc.tile_pool(name="const", bufs=1))
        w_pool = ctx.enter_context(tc.tile_pool(name="wp", bufs=1))
        x_pool = ctx.enter_context(tc.tile_pool(name="xp", bufs=4))
        qt_pool = ctx.enter_context(tc.tile_pool(name="qtp", bufs=4))
        kt_pool = ctx.enter_context(tc.tile_pool(name="ktp", bufs=4))
        v4_pool = ctx.enter_context(tc.tile_pool(name="v4p", bufs=4))
        vt_pool = ctx.enter_context(tc.tile_pool(name="vtp", bufs=1))
        pt_pool = ctx.enter_context(tc.tile_pool(name="ptp", bufs=3))
        cx_pool = ctx.enter_context(tc.tile_pool(name="cxp", bufs=16))
        rope_pool = ctx.enter_context(tc.tile_pool(name="ropep", bufs=2))
        rb_pool = ctx.enter_context(tc.tile_pool(name="rbp", bufs=2))
        sr_pool = ctx.enter_context(tc.tile_pool(name="srp", bufs=2))
        o_pool = ctx.enter_context(tc.tile_pool(name="op", bufs=4))
        cs_pool = ctx.enter_context(tc.tile_pool(name="csp", bufs=2))
        ps_pool = ctx.enter_context(tc.tile_pool(name="ps", bufs=3, space="PSUM"))

        # ---- constants (scalar HWDGE queue) ----
        mask_sb = const_pool.tile([128, 896], F32, tag="mask")
        ones_sb = const_pool.tile([128, 2], BF16, tag="ones")
        ident_sb = const_pool.tile([128, 128], BF16, tag="ident")
        nc.scalar.dma_start(mask_sb[:], maskb[:, :])
        nc.scalar.dma_start(ones_sb[:], ones[:, :])
        make_identity(nc, ident_sb[:])

        # ---- resident weights (bf16, partition-major host layout) on the
        # gpsimd SWDGE queue so they never contend with x on sync ----
        wq_sb = w_pool.tile([128, KT * DQ], BF16, tag="wq")
        wk_sb = w_pool.tile([128, KT * HD], BF16, tag="wk")
        wv_sb = w_pool.tile([128, KT * HD], BF16, tag="wv")
        wo_sb = w_pool.tile([128, NET * DQ], BF16, tag="wo")
        for g in range(8):  # 4 k-tiles per transfer, k/v interleaved so the
            nc.gpsimd.dma_start(  # chunk-0 k-loop's deps arrive in k order
                wq_sb[:, g * 4 * DQ:(g + 1) * 4 * DQ],
                wq[:, g * 4 * DQ:(g + 1) * 4 * DQ],
            )
            nc.gpsimd.dma_start(
                wk_sb[:, g * 4 * HD:(g + 1) * 4 * HD],
                wk[:, g * 4 * HD:(g + 1) * 4 * HD],
            )
            nc.gpsimd.dma_start(
                wv_sb[:, g * 4 * HD:(g + 1) * 4 * HD],
                wv[:, g * 4 * HD:(g + 1) * 4 * HD],
            )

        kt_tiles = []      # kT chunk tiles [128, TC] (d x tokens), bf16
        v4_tiles = []      # packed vT tiles [128, TC] (tokens x d), bf16
        cx_tiles = {}      # (h, chunk) -> ctxT tile [128, TC], bf16

        for c in range(NCH):
            # ================= projections for token chunk c =================
            ps_q01 = ps_pool.tile([128, 2 * TC], F32, tag="pair",
                                  name=f"psq01_{c}")
            ps_q23 = ps_pool.tile([128, 2 * TC], F32, tag="pair",
                                  name=f"psq23_{c}")
            ps_kv = ps_pool.tile([128, 2 * TC], F32, tag="pair",
                                 name=f"pskv_{c}")
            for k in range(KT):
                xt = x_pool.tile([128, TC], BF16, tag="x", name=f"x_{c}_{k}")
                nc.sync.dma_start(
                    xt[:], xT[k * 128:(k + 1) * 128, c * TC:(c + 1) * TC]
                )
                first, last = k == 0, k == KT - 1
                for h in range(NH):
                    dst = ps_q01 if h < 2 else ps_q23
                    nc.tensor.matmul(
                        dst[:, (h % 2) * TC:(h % 2 + 1) * TC],
                        wq_sb[:, k * DQ + h * HD:k * DQ + (h + 1) * HD],
                        xt[:],
                        start=first, stop=last,
                    )
                nc.tensor.matmul(
                    ps_kv[:, 0:TC], wk_sb[:, k * HD:(k + 1) * HD], xt[:],
                    start=first, stop=last,
                )
                nc.tensor.matmul(
                    ps_kv[:, TC:2 * TC], wv_sb[:, k * HD:(k + 1) * HD], xt[:],
                    start=first, stop=last,
                )

            if c == 0:  # wo after the chunk-0 weights on the same queue
                for g in range(4):
                    nc.gpsimd.dma_start(
                        wo_sb[:, g * 8 * DQ:(g + 1) * 8 * DQ],
                        wo[:, g * 8 * DQ:(g + 1) * 8 * DQ],
                    )

            cs_t = cs_pool.tile([HD, TC], BF16, tag="cos", name=f"cos_{c}")
            sn_t = cs_pool.tile([HD, TC], BF16, tag="sin", name=f"sin_{c}")
            nc.scalar.dma_start(cs_t[:], cosT[:, c * TC:(c + 1) * TC])
            nc.scalar.dma_start(sn_t[:], sinT[:, c * TC:(c + 1) * TC])
            cs = cs_t[:, :]
            sn = sn_t[:, :]

            def rope(ps_half, dst_pool, tag, nm):
                # psum-direct muls (crossed reads must come from PSUM — the
                # verifier requires SBUF operands to share start partitions),
                # bf16 outputs so the final add runs in the 2x DVE mode
                t1 = rope_pool.tile([128, TC], BF16, tag="t1", name=f"r1{nm}")
                t2 = rope_pool.tile([128, TC], BF16, tag="t2", name=f"r2{nm}")
                nc.vector.tensor_mul(t2[0:64, :], ps_half[64:128, :],
                                     sn[0:64, :])
                nc.vector.tensor_mul(t2[64:128, :], ps_half[0:64, :],
                                     sn[64:128, :])
                nc.vector.tensor_mul(t1[:], ps_half, cs)
                d = dst_pool.tile([128, TC], BF16, tag=tag, name=nm)
                nc.vector.tensor_add(d[:], t1[:], t2[:])
                return d

            # chunk 0's attention needs kt immediately; later chunks start
            # on old kt tiles and need q0/q1 first
            q_chunk = [None] * NH
            if c == 0:
                kt = rope(ps_kv[:, 0:TC], kt_pool, "kt", f"kt_{c}")
                q_chunk[0] = rope(ps_q01[:, 0:TC], qt_pool, "qt", f"qt_{c}_0")
                q_chunk[1] = rope(ps_q01[:, TC:2 * TC], qt_pool, "qt",
                                  f"qt_{c}_1")
            else:
                q_chunk[0] = rope(ps_q01[:, 0:TC], qt_pool, "qt", f"qt_{c}_0")
                q_chunk[1] = rope(ps_q01[:, TC:2 * TC], qt_pool, "qt",
                                  f"qt_{c}_1")
                kt = rope(ps_kv[:, 0:TC], kt_pool, "kt", f"kt_{c}")
            kt_tiles.append(kt)

            # v: ACT copy to bf16, 4 PE transposes into one packed psum tile,
            # one DVE cast out
            vt = vt_pool.tile([128, TC], BF16, tag="vt", name=f"vt_{c}")
            nc.vector.tensor_copy(vt[:], ps_kv[:, TC:2 * TC])
            ps_t = ps_pool.tile([128, TC], BF16, tag="pair",
                                name=f"pst_{c}")
            for jj in range(TC // 128):
                nc.tensor.transpose(ps_t[:, jj * 128:(jj + 1) * 128],
                                    vt[:, jj * 128:(jj + 1) * 128],
                                    ident_sb[:])
            v4 = v4_pool.tile([128, TC], BF16, tag="v4", name=f"v4_{c}")
            nc.vector.tensor_copy(v4[:], ps_t[:])
            v4_tiles.append(v4)

            q_chunk[2] = rope(ps_q23[:, 0:TC], qt_pool, "qt", f"qt_{c}_2")
            q_chunk[3] = rope(ps_q23[:, TC:2 * TC], qt_pool, "qt", f"qt_{c}_3")

            # ========== attention for i-chunk I = c, two heads at a time =====
            # Each head's rowsum accumulation group gets its OWN psum bank
            # (start=True clears has_written state per bank; sharing a bank
            # between groups accumulates onto stale data).
            I = c
            nj = 4 * I + 4
            ctx_un = {}
            srh = {}
            for hp in range(NH // 2):
                h0, h1 = 2 * hp, 2 * hp + 1
                ps_ctx = ps_pool.tile([128, 2 * TC], F32, tag="pair",
                                      name=f"psctx_{I}_{hp}")
                # per-head rowsum groups in their OWN banks (start=True
                # clears has_written state; groups must not share a bank)
                ps_s0 = ps_pool.tile([2, TC], F32, tag="one", bufs=2,
                                     name=f"pssum_{I}_{h0}")
                ps_s1 = ps_pool.tile([2, TC], F32, tag="one", bufs=2,
                                     name=f"pssum_{I}_{h1}")
                pts = {}

                def ctx_ones(J):
                    # Fully-masked i-subtiles (i-block < q) are skipped, so
                    # diagonal tiles write only cols [q*128, TC). Per column
                    # subtile s the last writer is diagonal J = 4I + s, which
                    # must carry its stop flag — hence the split matmuls.
                    first = J == 0
                    q = J - 4 * I
                    pt2 = pts.pop(J)
                    vst = v4_tiles[J // 4][:, (J % 4) * 128:(J % 4 + 1) * 128]
                    if q < 0:  # off-diagonal: full width, never a last writer
                        nc.tensor.matmul(ps_ctx[:, 0:TC], vst, pt2[:, 0:TC],
                                         start=first, stop=False)
                        nc.tensor.matmul(ps_ctx[:, TC:2 * TC], vst,
                                         pt2[:, TC:2 * TC],
                                         start=first, stop=False)
                        nc.tensor.matmul(ps_s0[:], ones_sb[:], pt2[:, 0:TC],
                                         start=first, stop=False)
                        nc.tensor.matmul(ps_s1[:], ones_sb[:],
                                         pt2[:, TC:2 * TC],
                                         start=first, stop=False)
                        return
                    w0, w1 = q * 128, (q + 1) * 128
                    for base in (0, TC):
                        nc.tensor.matmul(ps_ctx[:, base + w0:base + w1], vst,
                                         pt2[:, base + w0:base + w1],
                                         start=first, stop=True)
                        if w1 < TC:
                            nc.tensor.matmul(ps_ctx[:, base + w1:base + TC],
                                             vst, pt2[:, base + w1:base + TC],
                                             start=first, stop=False)
                    for ps_s, base in ((ps_s0, 0), (ps_s1, TC)):
                        nc.tensor.matmul(ps_s[:, w0:w1], ones_sb[:],
                                         pt2[:, base + w0:base + w1],
                                         start=first, stop=True)
                        if w1 < TC:
                            nc.tensor.matmul(ps_s[:, w1:TC], ones_sb[:],
                                             pt2[:, base + w1:base + TC],
                                             start=first, stop=False)

                for J in range(nj):
                    s2 = ps_pool.tile([128, 2 * TC], F32, tag="pair",
                                      name=f"pss_{I}_{hp}_{J}")
                    kst = kt_tiles[J // 4][:, (J % 4) * 128:(J % 4 + 1) * 128]
                    q = J - 4 * I
                    if q < 0:  # off-diagonal: full query range
                        nc.tensor.matmul(s2[:, 0:TC], kst, q_chunk[h0][:],
                                         start=True, stop=True)
                        nc.tensor.matmul(s2[:, TC:2 * TC], kst,
                                         q_chunk[h1][:],
                                         start=True, stop=True)
                        e0 = 0
                    else:
                        # diagonal: skip fully-masked i-subtiles, mask only
                        # the [128,128] triangle block (constant mask slice)
                        w0 = q * 128
                        nc.tensor.matmul(s2[:, w0:TC], kst,
                                         q_chunk[h0][:, w0:TC],
                                         start=True, stop=True)
                        nc.tensor.matmul(s2[:, TC + w0:2 * TC], kst,
                                         q_chunk[h1][:, w0:TC],
                                         start=True, stop=True)
                        nc.vector.tensor_add(
                            s2[:, w0:w0 + 128], s2[:, w0:w0 + 128],
                            mask_sb[:, 384:512]
                        )
                        nc.vector.tensor_add(
                            s2[:, TC + w0:TC + w0 + 128],
                            s2[:, TC + w0:TC + w0 + 128],
                            mask_sb[:, 384:512]
                        )
                        e0 = w0
                    pt2 = pt_pool.tile([128, 2 * TC], BF16, tag="pt",
                                       name=f"pt_{I}_{hp}_{J}")
                    nc.scalar.activation(pt2[:, e0:2 * TC], s2[:, e0:2 * TC],
                                         EXPF, scale=SCALE)
                    pts[J] = pt2
                    if J >= 1:
                        ctx_ones(J - 1)
                ctx_ones(nj - 1)

                # evacuate: rowsum copies (frees the sum banks for the next
                # head pair) + immediate [1,TC] reciprocal, ctx halves on
                # DVE/ACT
                for h, ps_s in ((h0, ps_s0), (h1, ps_s1)):
                    sl = sr_pool.tile([1, TC], F32, tag="sl", bufs=6,
                                      name=f"sl_{I}_{h}")
                    nc.scalar.copy(sl[:], ps_s[0:1, :])
                    nc.vector.reciprocal(sl[:], sl[:])
                    srh[h] = sl
                cx0 = cx_pool.tile([128, TC], BF16, tag="cx",
                                   name=f"cx_{I}_{h0}")
                nc.vector.tensor_copy(cx0[:], ps_ctx[:, 0:TC])
                ctx_un[h0] = cx0
                cx1 = cx_pool.tile([128, TC], BF16, tag="cx",
                                   name=f"cx_{I}_{h1}")
                nc.vector.tensor_copy(cx1[:], ps_ctx[:, TC:2 * TC])
                ctx_un[h1] = cx1

            # per-head broadcast + scale — off the psum path
            for h in range(NH):
                rb = rb_pool.tile([128, TC], F32, tag="rb", name=f"rb_{I}_{h}")
                nc.gpsimd.partition_broadcast(rb[:], srh[h][:])
                cxt = ctx_un[h]
                nc.vector.tensor_mul(cxt[:], cxt[:], rb[:])
                cx_tiles[(h, I)] = cxt

        # ======= output stage: outT[e,t], resident wo stationary tiles =======
        for Et in range(NET):
            ps_o = [ps_pool.tile([128, 2 * TC], F32, tag="pair",
                                 name=f"pso_{Et}_{p}") for p in range(2)]
            for h in range(NH):
                wst = wo_sb[:, Et * DQ + h * HD:Et * DQ + (h + 1) * HD]
                for tc_ in range(NCH):
                    nc.tensor.matmul(
                        ps_o[tc_ // 2][:, (tc_ % 2) * TC:(tc_ % 2 + 1) * TC],
                        wst,
                        cx_tiles[(h, tc_)][:],
                        start=h == 0, stop=h == NH - 1,
                    )
            for p in range(2):
                ot = o_pool.tile([128, 2 * TC], BF16, tag="o",
                                 name=f"o_{Et}_{p}")
                if p == 0:
                    nc.vector.tensor_copy(ot[:], ps_o[p][:])
                else:
                    nc.scalar.copy(ot[:], ps_o[p][:])
                nc.sync.dma_start(
                    out[Et * 128:(Et + 1) * 128,
                        p * 2 * TC:(p + 1) * 2 * TC],
                    ot[:],
                )


# ---------------------------------------------------------------------------
# host side
# ---------------------------------------------------------------------------
_NC_CACHE = None


def _get_nc():
    global _NC_CACHE
    if _NC_CACHE is None:
        _NC_CACHE = build_nc()
    return _NC_CACHE


def _pmajor(w, kt, width):
    """[kt*128, width] -> partition-major [128, kt*width] bf16."""
    return np.ascontiguousarray(
        w.reshape(kt, 128, width).transpose(1, 0, 2).reshape(128, kt * width)
    )


def make_in_maps(x, Wq, Wk, Wv, Wo, cos, sin):
    x = np.asarray(x, dtype=np.float32)
    xT = np.ascontiguousarray(x.reshape(T, D).T.astype(NPBF16))
    cosT = np.ascontiguousarray(
        np.asarray(cos, np.float32)[:T].T.astype(NPBF16))
    sin_t = np.asarray(sin, np.float32)[:T]          # [T, 128]
    sinT = sin_t.T.copy()                            # [128, T]
    sinT[:64] *= -1.0                                # fold rotate-half sign
    sinT = np.ascontiguousarray(sinT.astype(NPBF16))

    # sliding additive causal mask: diagonal tile q reads cols
    # (3-q)*128 : (3-q)*128 + (q+1)*128 of big[r, cc] = 0 if cc >= 384 + r
    r = np.arange(128)[:, None]
    cc = np.arange(896)[None, :]
    m = np.where(cc >= 384 + r, 0.0, -1.0e30).astype(np.float32)
    ones = np.ones((128, 2), NPBF16)

    Wq = np.asarray(Wq, np.float32).astype(NPBF16)
    Wk = np.asarray(Wk, np.float32).astype(NPBF16)
    Wv = np.asarray(Wv, np.float32).astype(NPBF16)
    Wo = np.asarray(Wo, np.float32).astype(NPBF16)
    in_maps = []
    for g in range(NCORES):
        # wo rows [g*DQ:(g+1)*DQ] shuffled to [dh, (Et, h, e)] so Et-tiles are
        # resident stationary slices of one partition-major tensor
        w = Wo[g * DQ:(g + 1) * DQ, :]                          # [512, 4096]
        w4 = w.reshape(NH, HD, NET, 128).transpose(1, 2, 0, 3)  # [dh,Et,h,e]
        woP = np.ascontiguousarray(w4.reshape(128, NET * DQ))
        in_maps.append({
            "xT": xT,
            "wq": _pmajor(Wq[:, g * DQ:(g + 1) * DQ], KT, DQ),
            "wk": _pmajor(Wk[:, g * HD:(g + 1) * HD], KT, HD),
            "wv": _pmajor(Wv[:, g * HD:(g + 1) * HD], KT, HD),
            "wo": woP,
            "cosT": cosT,
            "sinT": sinT,
            "maskb": m,
            "ones": ones,
        })
    return in_maps


def kernel(x, Wq, Wk, Wv, Wo, cos, sin):
    nc = _get_nc()
    in_maps = make_in_maps(x, Wq, Wk, Wv, Wo, cos, sin)
    res = run_bass_kernel_spmd(nc, in_maps, core_ids=list(range(NCORES)))
    acc = np.zeros((D, T), np.float32)
    for c in range(NCORES):
        acc += res.results[c]["out"].astype(np.float32)
    return np.ascontiguousarray(acc.T).reshape(1, T, D)


# revision 30
# speedup vs baseline: 1.5144x; 1.0271x over previous
"""GQA attention (32 heads, 8 KV groups, rope, causal) on 8 TRN2 NeuronCores.

Sharding: tensor-parallel over KV groups — core g owns KV group g
(4 query heads + 1 kv head). Wq/Wk/Wv sharded column-wise, Wo row-wise;
each core produces a partial transposed output outT=[D,T] in bf16,
summed in fp32 and transposed on the host.

Per-core dataflow (T=2048 tokens, D=4096, head_dim=128), all matmuls
bf16 with fp32 PSUM:
  proj:  pair-psums q01/q23/kv accumulate over 32 k-tiles; psum halves
         are ACT-evacuated to bf16 and rope runs as 4 bf16 DVE ops.
         v is PE-transposed (4 blocks into one packed psum) to tokens-major.
  attn (chunk I = 512 queries, 2 heads at a time, software-pipelined):
         S-pair[j, i|i'] = kt_J @ (q_h0|q_h1)  -> masked adds (DVE, width-
         restricted) -> one exp (ACT) -> pt pair (bf16)
         ctx-pair += v_J^T @ pt halves;  rowsums via ones-matmuls into one
         psum bank at partition 0 / 32 (col-group pair).
         Normalization: batched DVE reciprocal of the 4 rowsum rows per
         chunk, gpsimd partition-broadcast, DVE multiply.
  out:   resident wo (bf16) stationary tiles; paired [128,1024] psum,
         paired copies and 2KB-line DMAs.
DMA queues: x + out on sync HWDGE, weights (wq/wk/wv/wo) on gpsimd SWDGE,
constants on scalar HWDGE.
"""
import math

import ml_dtypes
import numpy as np

import concourse.bass as bass
import concourse.tile as tile
from concourse import bacc, mybir
from concourse.bass_utils import run_bass_kernel_spmd
from concourse.masks import make_identity

F32 = mybir.dt.float32
BF16 = mybir.dt.bfloat16
NPBF16 = ml_dtypes.bfloat16

T = 2048          # tokens
D = 4096          # model dim
HD = 128          # head dim
NH = 4            # heads per core
DQ = NH * HD      # 512 q dims per core
TC = 512          # token chunk (psum free dim)
NCH = T // TC     # 4 chunks
KT = D // 128     # 32 contraction tiles
JT = T // 128     # 16 key tiles
NET = D // 128    # 32 output-row tiles (of outT)
SCALE = 1.0 / math.sqrt(HD)
NCORES = 8
EXPF = mybir.ActivationFunctionType.Exp


def build_nc():
    nc = bacc.Bacc("TRN2", target_bir_lowering=False, debug=False, num_devices=NCORES)
    xT = nc.dram_tensor("xT", [D, T], BF16, kind="ExternalInput").ap()
    wq = nc.dram_tensor("wq", [128, KT * DQ], BF16, kind="ExternalInput").ap()
    wk = nc.dram_tensor("wk", [128, KT * HD], BF16, kind="ExternalInput").ap()
    wv = nc.dram_tensor("wv", [128, KT * HD], BF16, kind="ExternalInput").ap()
    wo = nc.dram_tensor("wo", [128, NET * DQ], BF16, kind="ExternalInput").ap()
    cosT = nc.dram_tensor("cosT", [HD, T], BF16, kind="ExternalInput").ap()
    sinT = nc.dram_tensor("sinT", [HD, T], BF16, kind="ExternalInput").ap()
    maskb = nc.dram_tensor("maskb", [128, 896], F32, kind="ExternalInput").ap()
    ones = nc.dram_tensor("ones", [128, 2], BF16, kind="ExternalInput").ap()
    out = nc.dram_tensor("out", [D, T], BF16, kind="ExternalOutput").ap()

    with tile.TileContext(nc) as tc:
        _body(tc, out, xT, wq, wk, wv, wo, cosT, sinT, maskb, ones)
    nc.compile()
    return nc


def _body(tc, out, xT, wq, wk, wv, wo, cosT, sinT, maskb, ones):
    nc = tc.nc
    from contextlib import ExitStack

    with ExitStack() as ctx:
        const_pool = ctx.enter_context(tc.tile_pool(name="const", bufs=1))
        w_pool = ctx.enter_context(tc.tile_pool(name="wp", bufs=1))
        x_pool = ctx.enter_context(tc.tile_pool(name="xp", bufs=4))
        qt_pool = ctx.enter_context(tc.tile_pool(name="qtp", bufs=4))
        kt_pool = ctx.enter_context(tc.tile_pool(name="ktp", bufs=4))
        v4_pool = ctx.enter_context(tc.tile_pool(name="v4p", bufs=4))
        vt_pool = ctx.enter_context(tc.tile_pool(name="vtp", bufs=1))
        pt_pool = ctx.enter_context(tc.tile_pool(name="ptp", bufs=3))
        cx_pool = ctx.enter_context(tc.tile_pool(name="cxp", bufs=16))
        rope_pool = ctx.enter_context(tc.tile_pool(name="ropep", bufs=2))
        rb_pool = ctx.enter_context(tc.tile_pool(name="rbp", bufs=2))
        sr_pool = ctx.enter_context(tc.tile_pool(name="srp", bufs=2))
        o_pool = ctx.enter_context(tc.tile_pool(name="op", bufs=4))
        cs_pool = ctx.enter_context(tc.tile_pool(name="csp", bufs=2))
        ps_pool = ctx.enter_context(tc.tile_pool(name="ps", bufs=3, space="PSUM"))

        # ---- constants (scalar HWDGE queue) ----
        mask_sb = const_pool.tile([128, 896], F32, tag="mask")
        ones_sb = const_pool.tile([128, 2], BF16, tag="ones")
        ident_sb = const_pool.tile([128, 128], BF16, tag="ident")
        nc.scalar.dma_start(mask_sb[:], maskb[:, :])
        nc.scalar.dma_start(ones_sb[:], ones[:, :])
        make_identity(nc, ident_sb[:])

        # ---- resident weights (bf16, partition-major host layout) on the
        # gpsimd SWDGE queue so they never contend with x on sync ----
        wq_sb = w_pool.tile([128, KT * DQ], BF16, tag="wq")
        wk_sb = w_pool.tile([128, KT * HD], BF16, tag="wk")
        wv_sb = w_pool.tile([128, KT * HD], BF16, tag="wv")
        wo_sb = w_pool.tile([128, NET * DQ], BF16, tag="wo")
        for g in range(8):  # 4 k-tiles per transfer, k/v interleaved so the
            nc.gpsimd.dma_start(  # chunk-0 k-loop's deps arrive in k order
                wq_sb[:, g * 4 * DQ:(g + 1) * 4 * DQ],
                wq[:, g * 4 * DQ:(g + 1) * 4 * DQ],
            )
            nc.gpsimd.dma_start(
                wk_sb[:, g * 4 * HD:(g + 1) * 4 * HD],
                wk[:, g * 4 * HD:(g + 1) * 4 * HD],
            )
            nc.gpsimd.dma_start(
                wv_sb[:, g * 4 * HD:(g + 1) * 4 * HD],
                wv[:, g * 4 * HD:(g + 1) * 4 * HD],
            )

        kt_tiles = []      # kT chunk tiles [128, TC] (d x tokens), bf16
        v4_tiles = []      # packed vT tiles [128, TC] (tokens x d), bf16
        cx_tiles = {}      # (h, chunk) -> ctxT tile [128, TC], bf16

        for c in range(NCH):
            # ================= projections for token chunk c =================
            ps_q01 = ps_pool.tile([128, 2 * TC], F32, tag="pair",
                                  name=f"psq01_{c}")
            ps_q23 = ps_pool.tile([128, 2 * TC], F32, tag="pair",
                                  name=f"psq23_{c}")
            ps_kv = ps_pool.tile([128, 2 * TC], F32, tag="pair",
                                 name=f"pskv_{c}")
            for k in range(KT):
                xt = x_pool.tile([128, TC], BF16, tag="x", name=f"x_{c}_{k}")
                nc.sync.dma_start(
                    xt[:], xT[k * 128:(k + 1) * 128, c * TC:(c + 1) * TC]
                )
                first, last = k == 0, k == KT - 1
                for h in range(NH):
                    dst = ps_q01 if h < 2 else ps_q23
                    nc.tensor.matmul(
                        dst[:, (h % 2) * TC:(h % 2 + 1) * TC],
                        wq_sb[:, k * DQ + h * HD:k * DQ + (h + 1) * HD],
                        xt[:],
                        start=first, stop=last,
                    )
                nc.tensor.matmul(
                    ps_kv[:, 0:TC], wk_sb[:, k * HD:(k + 1) * HD], xt[:],
                    start=first, stop=last,
                )
                nc.tensor.matmul(
                    ps_kv[:, TC:2 * TC], wv_sb[:, k * HD:(k + 1) * HD], xt[:],
                    start=first, stop=last,
                )

            if c == 0:  # wo after the chunk-0 weights on the same queue
                for g in range(4):
                    nc.gpsimd.dma_start(
                        wo_sb[:, g * 8 * DQ:(g + 1) * 8 * DQ],
                        wo[:, g * 8 * DQ:(g + 1) * 8 * DQ],
                    )

            cs_t = cs_pool.tile([HD, TC], BF16, tag="cos", name=f"cos_{c}")
            sn_t = cs_pool.tile([HD, TC], BF16, tag="sin", name=f"sin_{c}")
            nc.scalar.dma_start(cs_t[:], cosT[:, c * TC:(c + 1) * TC])
            nc.scalar.dma_start(sn_t[:], sinT[:, c * TC:(c + 1) * TC])
            cs = cs_t[:, :]
            sn = sn_t[:, :]

            def rope(ps_half, dst_pool, tag, nm):
                # psum-direct muls (crossed reads must come from PSUM — the
                # verifier requires SBUF operands to share start partitions),
                # bf16 outputs so the final add runs in the 2x DVE mode
                t1 = rope_pool.tile([128, TC], BF16, tag="t1", name=f"r1{nm}")
                t2 = rope_pool.tile([128, TC], BF16, tag="t2", name=f"r2{nm}")
                nc.vector.tensor_mul(t2[0:64, :], ps_half[64:128, :],
                                     sn[0:64, :])
                nc.vector.tensor_mul(t2[64:128, :], ps_half[0:64, :],
                                     sn[64:128, :])
                nc.vector.tensor_mul(t1[:], ps_half, cs)
                d = dst_pool.tile([128, TC], BF16, tag=tag, name=nm)
                nc.vector.tensor_add(d[:], t1[:], t2[:])
                return d

            # chunk 0's attention needs kt immediately; later chunks start
            # on old kt tiles and need q0/q1 first
            q_chunk = [None] * NH
            if c == 0:
                kt = rope(ps_kv[:, 0:TC], kt_pool, "kt", f"kt_{c}")
                q_chunk[0] = rope(ps_q01[:, 0:TC], qt_pool, "qt", f"qt_{c}_0")
                q_chunk[1] = rope(ps_q01[:, TC:2 * TC], qt_pool, "qt",
                                  f"qt_{c}_1")
            else:
                q_chunk[0] = rope(ps_q01[:, 0:TC], qt_pool, "qt", f"qt_{c}_0")
                q_chunk[1] = rope(ps_q01[:, TC:2 * TC], qt_pool, "qt",
                                  f"qt_{c}_1")
                kt = rope(ps_kv[:, 0:TC], kt_pool, "kt", f"kt_{c}")
            kt_tiles.append(kt)

            # v: ACT copy to bf16, 4 PE transposes into one packed psum tile,
            # one DVE cast out
            vt = vt_pool.tile([128, TC], BF16, tag="vt", name=f"vt_{c}")
            nc.vector.tensor_copy(vt[:], ps_kv[:, TC:2 * TC])
            ps_t = ps_pool.tile([128, TC], BF16, tag="pair",
                                name=f"pst_{c}")
            for jj in range(TC // 128):
                nc.tensor.transpose(ps_t[:, jj * 128:(jj + 1) * 128],
                                    vt[:, jj * 128:(jj + 1) * 128],
                                    ident_sb[:])
            v4 = v4_pool.tile([128, TC], BF16, tag="v4", name=f"v4_{c}")
            nc.vector.tensor_copy(v4[:], ps_t[:])
            v4_tiles.append(v4)

            q_chunk[2] = rope(ps_q23[:, 0:TC], qt_pool, "qt", f"qt_{c}_2")
            q_chunk[3] = rope(ps_q23[:, TC:2 * TC], qt_pool, "qt", f"qt_{c}_3")

            # ========== attention for i-chunk I = c, two heads at a time =====
            # Each head's rowsum accumulation group gets its OWN psum bank
            # (start=True clears has_written state per bank; sharing a bank
            # between groups accumulates onto stale data).
            I = c
            nj = 4 * I + 4
            ctx_un = {}
            srh = {}
            for hp in range(NH // 2):
                h0, h1 = 2 * hp, 2 * hp + 1
                ps_ctx = ps_pool.tile([128, 2 * TC], F32, tag="pair",
                                      name=f"psctx_{I}_{hp}")
                # per-head rowsum groups in their OWN banks (start=True
                # clears has_written state; groups must not share a bank)
                ps_s0 = ps_pool.tile([2, TC], F32, tag="one", bufs=2,
                                     name=f"pssum_{I}_{h0}")
                ps_s1 = ps_pool.tile([2, TC], F32, tag="one", bufs=2,
                                     name=f"pssum_{I}_{h1}")
                pts = {}

                def ctx_ones(J):
                    # Fully-masked i-subtiles (i-block < q) are skipped, so
                    # diagonal tiles write only cols [q*128, TC). Per column
                    # subtile s the last writer is diagonal J = 4I + s, which
                    # must carry its stop flag — hence the split matmuls.
                    first = J == 0
                    q = J - 4 * I
                    pt2 = pts.pop(J)
                    vst = v4_tiles[J // 4][:, (J % 4) * 128:(J % 4 + 1) * 128]
                    if q < 0:  # off-diagonal: full width, never a last writer
                        nc.tensor.matmul(ps_ctx[:, 0:TC], vst, pt2[:, 0:TC],
                                         start=first, stop=False)
                        nc.tensor.matmul(ps_ctx[:, TC:2 * TC], vst,
                                         pt2[:, TC:2 * TC],
                                         start=first, stop=False)
                        nc.tensor.matmul(ps_s0[:], ones_sb[:], pt2[:, 0:TC],
                                         start=first, stop=False)
                        nc.tensor.matmul(ps_s1[:], ones_sb[:],
                                         pt2[:, TC:2 * TC],
                                         start=first, stop=False)
                        return
                    w0, w1 = q * 128, (q + 1) * 128
                    for base in (0, TC):
                        nc.tensor.matmul(ps_ctx[:, base + w0:base + w1], vst,
                                         pt2[:, base + w0:base + w1],
                                         start=first, stop=True)
                        if w1 < TC:
                            nc.tensor.matmul(ps_ctx[:, base + w1:base + TC],
                                             vst, pt2[:, base + w1:base + TC],
                                             start=first, stop=False)
                    for ps_s, base in ((ps_s0, 0), (ps_s1, TC)):
                        nc.tensor.matmul(ps_s[:, w0:w1], ones_sb[:],
                                         pt2[:, base + w0:base + w1],
                                         start=first, stop=True)
                        if w1 < TC:
                            nc.tensor.matmul(ps_s[:, w1:TC], ones_sb[:],
                                             pt2[:, base + w1:base + TC],
                                             start=first, stop=False)

                for J in range(nj):
                    s2 = ps_pool.tile([128, 2 * TC], F32, tag="pair",
                                      name=f"pss_{I}_{hp}_{J}")
                    kst = kt_tiles[J // 4][:, (J % 4) * 128:(J % 4 + 1) * 128]
                    q = J - 4 * I
                    if q < 0:  # off-diagonal: full query range
                        nc.tensor.matmul(s2[:, 0:TC], kst, q_chunk[h0][:],
                                         start=True, stop=True)
                        nc.tensor.matmul(s2[:, TC:2 * TC], kst,
                                         q_chunk[h1][:],
                                         start=True, stop=True)
                        e0 = 0
                    else:
                        # diagonal: skip fully-masked i-subtiles, mask only
                        # the [128,128] triangle block (constant mask slice)
                        w0 = q * 128
                        nc.tensor.matmul(s2[:, w0:TC], kst,
                                         q_chunk[h0][:, w0:TC],
                                         start=True, stop=True)
                        nc.tensor.matmul(s2[:, TC + w0:2 * TC], kst,
                                         q_chunk[h1][:, w0:TC],
                                         start=True, stop=True)
                        nc.vector.tensor_add(
                            s2[:, w0:w0 + 128], s2[:, w0:w0 + 128],
                            mask_sb[:, 384:512]
                        )
                        nc.vector.tensor_add(
                            s2[:, TC + w0:TC + w0 + 128],
                            s2[:, TC + w0:TC + w0 + 128],
                            mask_sb[:, 384:512]
                        )
                        e0 = w0
                    pt2 = pt_pool.tile([128, 2 * TC], BF16, tag="pt",
                                       name=f"pt_{I}_{hp}_{J}")
                    nc.scalar.activation(pt2[:, e0:2 * TC], s2[:, e0:2 * TC],
                                         EXPF, scale=SCALE)
                    pts[J] = pt2
                    if J >= 1:
                        ctx_ones(J - 1)
                ctx_ones(nj - 1)

                # evacuate: ctx halves first (they gate the next chunk's
                # psum slots), then rowsum copies + fast [1,TC] reciprocals
                cx0 = cx_pool.tile([128, TC], BF16, tag="cx",
                                   name=f"cx_{I}_{h0}")
                nc.vector.tensor_copy(cx0[:], ps_ctx[:, 0:TC])
                ctx_un[h0] = cx0
                cx1 = cx_pool.tile([128, TC], BF16, tag="cx",
                                   name=f"cx_{I}_{h1}")
                nc.vector.tensor_copy(cx1[:], ps_ctx[:, TC:2 * TC])
                ctx_un[h1] = cx1
                for h, ps_s in ((h0, ps_s0), (h1, ps_s1)):
                    sl = sr_pool.tile([1, TC], F32, tag="sl", bufs=6,
                                      name=f"sl_{I}_{h}")
                    nc.scalar.copy(sl[:], ps_s[0:1, :])
                    nc.vector.reciprocal_approx_fast(sl[:], sl[:])
                    srh[h] = sl

            # per-head broadcast + scale — off the psum path
            for h in range(NH):
                rb = rb_pool.tile([128, TC], F32, tag="rb", name=f"rb_{I}_{h}")
                nc.gpsimd.partition_broadcast(rb[:], srh[h][:])
                cxt = ctx_un[h]
                nc.vector.tensor_mul(cxt[:], cxt[:], rb[:])
                cx_tiles[(h, I)] = cxt

        # ======= output stage: outT[e,t], resident wo stationary tiles =======
        for Et in range(NET):
            ps_o = [ps_pool.tile([128, 2 * TC], F32, tag="pair",
                                 name=f"pso_{Et}_{p}") for p in range(2)]
            for h in range(NH):
                wst = wo_sb[:, Et * DQ + h * HD:Et * DQ + (h + 1) * HD]
                for tc_ in range(NCH):
                    nc.tensor.matmul(
                        ps_o[tc_ // 2][:, (tc_ % 2) * TC:(tc_ % 2 + 1) * TC],
                        wst,
                        cx_tiles[(h, tc_)][:],
                        start=h == 0, stop=h == NH - 1,
                    )
            for p in range(2):
                ot = o_pool.tile([128, 2 * TC], BF16, tag="o",
                                 name=f"o_{Et}_{p}")
                if p == 0:
                    nc.vector.tensor_copy(ot[:], ps_o[p][:])
                else:
                    nc.scalar.copy(ot[:], ps_o[p][:])
                nc.sync.dma_start(
                    out[Et * 128:(Et + 1) * 128,
                        p * 2 * TC:(p + 1) * 2 * TC],
                    ot[:],
                )


# ---------------------------------------------------------------------------
# host side
# ---------------------------------------------------------------------------
_NC_CACHE = None


def _get_nc():
    global _NC_CACHE
    if _NC_CACHE is None:
        _NC_CACHE = build_nc()
    return _NC_CACHE


def _pmajor(w, kt, width):
    """[kt*128, width] -> partition-major [128, kt*width] bf16."""
    return np.ascontiguousarray(
        w.reshape(kt, 128, width).transpose(1, 0, 2).reshape(128, kt * width)
    )


def make_in_maps(x, Wq, Wk, Wv, Wo, cos, sin):
    x = np.asarray(x, dtype=np.float32)
    xT = np.ascontiguousarray(x.reshape(T, D).T.astype(NPBF16))
    cosT = np.ascontiguousarray(
        np.asarray(cos, np.float32)[:T].T.astype(NPBF16))
    sin_t = np.asarray(sin, np.float32)[:T]          # [T, 128]
    sinT = sin_t.T.copy()                            # [128, T]
    sinT[:64] *= -1.0                                # fold rotate-half sign
    sinT = np.ascontiguousarray(sinT.astype(NPBF16))

    # sliding additive causal mask: diagonal tile q reads cols
    # (3-q)*128 : (3-q)*128 + (q+1)*128 of big[r, cc] = 0 if cc >= 384 + r
    r = np.arange(128)[:, None]
    cc = np.arange(896)[None, :]
    m = np.where(cc >= 384 + r, 0.0, -1.0e30).astype(np.float32)
    ones = np.ones((128, 2), NPBF16)

    Wq = np.asarray(Wq, np.float32).astype(NPBF16)
    Wk = np.asarray(Wk, np.float32).astype(NPBF16)
    Wv = np.asarray(Wv, np.float32).astype(NPBF16)
    Wo = np.asarray(Wo, np.float32).astype(NPBF16)
    in_maps = []
    for g in range(NCORES):
        # wo rows [g*DQ:(g+1)*DQ] shuffled to [dh, (Et, h, e)] so Et-tiles are
        # resident stationary slices of one partition-major tensor
        w = Wo[g * DQ:(g + 1) * DQ, :]                          # [512, 4096]
        w4 = w.reshape(NH, HD, NET, 128).transpose(1, 2, 0, 3)  # [dh,Et,h,e]
        woP = np.ascontiguousarray(w4.reshape(128, NET * DQ))
        in_maps.append({
            "xT": xT,
            "wq": _pmajor(Wq[:, g * DQ:(g + 1) * DQ], KT, DQ),
            "wk": _pmajor(Wk[:, g * HD:(g + 1) * HD], KT, HD),
            "wv": _pmajor(Wv[:, g * HD:(g + 1) * HD], KT, HD),
            "wo": woP,
            "cosT": cosT,
            "sinT": sinT,
            "maskb": m,
            "ones": ones,
        })
    return in_maps


def kernel(x, Wq, Wk, Wv, Wo, cos, sin):
    nc = _get_nc()
    in_maps = make_in_maps(x, Wq, Wk, Wv, Wo, cos, sin)
    res = run_bass_kernel_spmd(nc, in_maps, core_ids=list(range(NCORES)))
    acc = np.zeros((D, T), np.float32)
    for c in range(NCORES):
        acc += res.results[c]["out"].astype(np.float32)
    return np.ascontiguousarray(acc.T).reshape(1, T, D)


# revision 33
# speedup vs baseline: 1.5458x; 1.0207x over previous
"""GQA attention (32 heads, 8 KV groups, rope, causal) on 8 TRN2 NeuronCores.

Sharding: tensor-parallel over KV groups — core g owns KV group g
(4 query heads + 1 kv head). Wq/Wk/Wv sharded column-wise, Wo row-wise;
each core produces a partial transposed output outT=[D,T] in bf16,
summed in fp32 and transposed on the host.

Per-core dataflow (T=2048 tokens, D=4096, head_dim=128), all matmuls
bf16 with fp32 PSUM:
  proj:  pair-psums q01/q23/kv accumulate over 32 k-tiles; psum halves
         are ACT-evacuated to bf16 and rope runs as 4 bf16 DVE ops.
         v is PE-transposed (4 blocks into one packed psum) to tokens-major.
  attn (chunk I = 512 queries, 2 heads at a time, software-pipelined):
         S-pair[j, i|i'] = kt_J @ (q_h0|q_h1)  -> masked adds (DVE, width-
         restricted) -> one exp (ACT) -> pt pair (bf16)
         ctx-pair += v_J^T @ pt halves;  rowsums via ones-matmuls into one
         psum bank at partition 0 / 32 (col-group pair).
         Normalization: batched DVE reciprocal of the 4 rowsum rows per
         chunk, gpsimd partition-broadcast, DVE multiply.
  out:   resident wo (bf16) stationary tiles; paired [128,1024] psum,
         paired copies and 2KB-line DMAs.
DMA queues: x + out on sync HWDGE, weights (wq/wk/wv/wo) on gpsimd SWDGE,
constants on scalar HWDGE.
"""
import math

import ml_dtypes
import numpy as np

import concourse.bass as bass
import concourse.tile as tile
from concourse import bacc, mybir
from concourse.bass_utils import run_bass_kernel_spmd
from concourse.masks import make_identity

F32 = mybir.dt.float32
BF16 = mybir.dt.bfloat16
NPBF16 = ml_dtypes.bfloat16

T = 2048          # tokens
D = 4096          # model dim
HD = 128          # head dim
NH = 4            # heads per core
DQ = NH * HD      # 512 q dims per core
TC = 512          # token chunk (psum free dim)
NCH = T // TC     # 4 chunks
KT = D // 128     # 32 contraction tiles
JT = T // 128     # 16 key tiles
NET = D // 128    # 32 output-row tiles (of outT)
SCALE = 1.0 / math.sqrt(HD)
NCORES = 8
EXPF = mybir.ActivationFunctionType.Exp


def build_nc():
    nc = bacc.Bacc("TRN2", target_bir_lowering=False, debug=False, num_devices=NCORES)
    xT = nc.dram_tensor("xT", [D, T], BF16, kind="ExternalInput").ap()
    wq = nc.dram_tensor("wq", [128, KT * DQ], BF16, kind="ExternalInput").ap()
    wk = nc.dram_tensor("wk", [128, KT * HD], BF16, kind="ExternalInput").ap()
    wv = nc.dram_tensor("wv", [128, KT * HD], BF16, kind="ExternalInput").ap()
    wo = nc.dram_tensor("wo", [128, NET * DQ], BF16, kind="ExternalInput").ap()
    cosT = nc.dram_tensor("cosT", [HD, T], BF16, kind="ExternalInput").ap()
    sinT = nc.dram_tensor("sinT", [HD, T], BF16, kind="ExternalInput").ap()
    maskb = nc.dram_tensor("maskb", [128, 896], F32, kind="ExternalInput").ap()
    ones = nc.dram_tensor("ones", [128, 2], BF16, kind="ExternalInput").ap()
    out = nc.dram_tensor("out", [D, T], BF16, kind="ExternalOutput").ap()

    with tile.TileContext(nc) as tc:
        _body(tc, out, xT, wq, wk, wv, wo, cosT, sinT, maskb, ones)
    nc.compile()
    return nc


def _body(tc, out, xT, wq, wk, wv, wo, cosT, sinT, maskb, ones):
    nc = tc.nc
    from contextlib import ExitStack

    with ExitStack() as ctx:
        const_pool = ctx.enter_context(tc.tile_pool(name="const", bufs=1))
        w_pool = ctx.enter_context(tc.tile_pool(name="wp", bufs=1))
        x_pool = ctx.enter_context(tc.tile_pool(name="xp", bufs=6))
        qt_pool = ctx.enter_context(tc.tile_pool(name="qtp", bufs=4))
        kt_pool = ctx.enter_context(tc.tile_pool(name="ktp", bufs=4))
        v4_pool = ctx.enter_context(tc.tile_pool(name="v4p", bufs=4))
        vt_pool = ctx.enter_context(tc.tile_pool(name="vtp", bufs=1))
        pt_pool = ctx.enter_context(tc.tile_pool(name="ptp", bufs=3))
        cx_pool = ctx.enter_context(tc.tile_pool(name="cxp", bufs=16))
        rope_pool = ctx.enter_context(tc.tile_pool(name="ropep", bufs=2))
        rb_pool = ctx.enter_context(tc.tile_pool(name="rbp", bufs=2))
        sr_pool = ctx.enter_context(tc.tile_pool(name="srp", bufs=2))
        o_pool = ctx.enter_context(tc.tile_pool(name="op", bufs=4))
        cs_pool = ctx.enter_context(tc.tile_pool(name="csp", bufs=2))
        ps_pool = ctx.enter_context(tc.tile_pool(name="ps", bufs=3, space="PSUM"))

        # ---- constants (scalar HWDGE queue) ----
        mask_sb = const_pool.tile([128, 896], F32, tag="mask")
        ones_sb = const_pool.tile([128, 2], BF16, tag="ones")
        ident_sb = const_pool.tile([128, 128], BF16, tag="ident")
        nc.scalar.dma_start(mask_sb[:], maskb[:, :])
        nc.scalar.dma_start(ones_sb[:], ones[:, :])
        make_identity(nc, ident_sb[:])

        # ---- resident weights (bf16, partition-major host layout) on the
        # gpsimd SWDGE queue so they never contend with x on sync ----
        wq_sb = w_pool.tile([128, KT * DQ], BF16, tag="wq")
        wk_sb = w_pool.tile([128, KT * HD], BF16, tag="wk")
        wv_sb = w_pool.tile([128, KT * HD], BF16, tag="wv")
        wo_sb = w_pool.tile([128, NET * DQ], BF16, tag="wo")
        for g in range(8):  # 4 k-tiles per transfer; wq on the gpsimd SWDGE
            nc.gpsimd.dma_start(  # queue, wk/wv on the scalar HWDGE queue so
                wq_sb[:, g * 4 * DQ:(g + 1) * 4 * DQ],  # they load in parallel
                wq[:, g * 4 * DQ:(g + 1) * 4 * DQ],
            )
            nc.scalar.dma_start(
                wk_sb[:, g * 4 * HD:(g + 1) * 4 * HD],
                wk[:, g * 4 * HD:(g + 1) * 4 * HD],
            )
            nc.scalar.dma_start(
                wv_sb[:, g * 4 * HD:(g + 1) * 4 * HD],
                wv[:, g * 4 * HD:(g + 1) * 4 * HD],
            )

        kt_tiles = []      # kT chunk tiles [128, TC] (d x tokens), bf16
        v4_tiles = []      # packed vT tiles [128, TC] (tokens x d), bf16
        cx_tiles = {}      # (h, chunk) -> ctxT tile [128, TC], bf16

        for c in range(NCH):
            # ================= projections for token chunk c =================
            ps_q01 = ps_pool.tile([128, 2 * TC], F32, tag="pair",
                                  name=f"psq01_{c}")
            ps_q23 = ps_pool.tile([128, 2 * TC], F32, tag="pair",
                                  name=f"psq23_{c}")
            ps_kv = ps_pool.tile([128, 2 * TC], F32, tag="pair",
                                 name=f"pskv_{c}")
            for k in range(KT):
                xt = x_pool.tile([128, TC], BF16, tag="x", name=f"x_{c}_{k}")
                nc.sync.dma_start(
                    xt[:], xT[k * 128:(k + 1) * 128, c * TC:(c + 1) * TC]
                )
                first, last = k == 0, k == KT - 1
                for h in range(NH):
                    dst = ps_q01 if h < 2 else ps_q23
                    nc.tensor.matmul(
                        dst[:, (h % 2) * TC:(h % 2 + 1) * TC],
                        wq_sb[:, k * DQ + h * HD:k * DQ + (h + 1) * HD],
                        xt[:],
                        start=first, stop=last,
                    )
                nc.tensor.matmul(
                    ps_kv[:, 0:TC], wk_sb[:, k * HD:(k + 1) * HD], xt[:],
                    start=first, stop=last,
                )
                nc.tensor.matmul(
                    ps_kv[:, TC:2 * TC], wv_sb[:, k * HD:(k + 1) * HD], xt[:],
                    start=first, stop=last,
                )

            if c == 0:  # wo after the chunk-0 weights on the same queue
                for g in range(4):
                    nc.gpsimd.dma_start(
                        wo_sb[:, g * 8 * DQ:(g + 1) * 8 * DQ],
                        wo[:, g * 8 * DQ:(g + 1) * 8 * DQ],
                    )

            cs_t = cs_pool.tile([HD, TC], BF16, tag="cos", name=f"cos_{c}")
            sn_t = cs_pool.tile([HD, TC], BF16, tag="sin", name=f"sin_{c}")
            nc.scalar.dma_start(cs_t[:], cosT[:, c * TC:(c + 1) * TC])
            nc.scalar.dma_start(sn_t[:], sinT[:, c * TC:(c + 1) * TC])
            cs = cs_t[:, :]
            sn = sn_t[:, :]

            def rope(ps_half, dst_pool, tag, nm):
                # psum-direct muls (crossed reads must come from PSUM — the
                # verifier requires SBUF operands to share start partitions),
                # bf16 outputs so the final add runs in the 2x DVE mode
                t1 = rope_pool.tile([128, TC], BF16, tag="t1", name=f"r1{nm}")
                t2 = rope_pool.tile([128, TC], BF16, tag="t2", name=f"r2{nm}")
                nc.vector.tensor_mul(t2[0:64, :], ps_half[64:128, :],
                                     sn[0:64, :])
                nc.vector.tensor_mul(t2[64:128, :], ps_half[0:64, :],
                                     sn[64:128, :])
                nc.vector.tensor_mul(t1[:], ps_half, cs)
                d = dst_pool.tile([128, TC], BF16, tag=tag, name=nm)
                nc.vector.tensor_add(d[:], t1[:], t2[:])
                return d

            # v-half evacuation first so the PE transposes aren't gated by
            # the rope chain on the DVE; then the ropes in need order
            # (chunk 0's attention needs kt immediately; later chunks start
            # on old kt tiles and need q0/q1 first)
            vt = vt_pool.tile([128, TC], BF16, tag="vt", name=f"vt_{c}")
            nc.vector.tensor_copy(vt[:], ps_kv[:, TC:2 * TC])
            q_chunk = [None] * NH
            if c == 0:
                kt = rope(ps_kv[:, 0:TC], kt_pool, "kt", f"kt_{c}")
                q_chunk[0] = rope(ps_q01[:, 0:TC], qt_pool, "qt", f"qt_{c}_0")
                q_chunk[1] = rope(ps_q01[:, TC:2 * TC], qt_pool, "qt",
                                  f"qt_{c}_1")
            else:
                q_chunk[0] = rope(ps_q01[:, 0:TC], qt_pool, "qt", f"qt_{c}_0")
                q_chunk[1] = rope(ps_q01[:, TC:2 * TC], qt_pool, "qt",
                                  f"qt_{c}_1")
                kt = rope(ps_kv[:, 0:TC], kt_pool, "kt", f"kt_{c}")
            kt_tiles.append(kt)

            ps_t = ps_pool.tile([128, TC], BF16, tag="pair",
                                name=f"pst_{c}")
            for jj in range(TC // 128):
                nc.tensor.transpose(ps_t[:, jj * 128:(jj + 1) * 128],
                                    vt[:, jj * 128:(jj + 1) * 128],
                                    ident_sb[:])
            v4 = v4_pool.tile([128, TC], BF16, tag="v4", name=f"v4_{c}")
            nc.vector.tensor_copy(v4[:], ps_t[:])
            v4_tiles.append(v4)

            q_chunk[2] = rope(ps_q23[:, 0:TC], qt_pool, "qt", f"qt_{c}_2")
            q_chunk[3] = rope(ps_q23[:, TC:2 * TC], qt_pool, "qt", f"qt_{c}_3")

            # ========== attention for i-chunk I = c, two heads at a time =====
            # Each head's rowsum accumulation group gets its OWN psum bank
            # (start=True clears has_written state per bank; sharing a bank
            # between groups accumulates onto stale data).
            I = c
            nj = 4 * I + 4
            ctx_un = {}
            srh = {}
            for hp in range(NH // 2):
                h0, h1 = 2 * hp, 2 * hp + 1
                ps_ctx = ps_pool.tile([128, 2 * TC], F32, tag="pair",
                                      name=f"psctx_{I}_{hp}")
                # per-head rowsum groups in their OWN banks (start=True
                # clears has_written state; groups must not share a bank)
                ps_s0 = ps_pool.tile([2, TC], F32, tag="one", bufs=2,
                                     name=f"pssum_{I}_{h0}")
                ps_s1 = ps_pool.tile([2, TC], F32, tag="one", bufs=2,
                                     name=f"pssum_{I}_{h1}")
                pts = {}

                def ctx_ones(J):
                    # Fully-masked i-subtiles (i-block < q) are skipped, so
                    # diagonal tiles write only cols [q*128, TC). Per column
                    # subtile s the last writer is diagonal J = 4I + s, which
                    # must carry its stop flag — hence the split matmuls.
                    first = J == 0
                    q = J - 4 * I
                    pt2 = pts.pop(J)
                    vst = v4_tiles[J // 4][:, (J % 4) * 128:(J % 4 + 1) * 128]
                    if q < 0:  # off-diagonal: full width, never a last writer
                        nc.tensor.matmul(ps_ctx[:, 0:TC], vst, pt2[:, 0:TC],
                                         start=first, stop=False)
                        nc.tensor.matmul(ps_ctx[:, TC:2 * TC], vst,
                                         pt2[:, TC:2 * TC],
                                         start=first, stop=False)
                        nc.tensor.matmul(ps_s0[:], ones_sb[:], pt2[:, 0:TC],
                                         start=first, stop=False)
                        nc.tensor.matmul(ps_s1[:], ones_sb[:],
                                         pt2[:, TC:2 * TC],
                                         start=first, stop=False)
                        return
                    w0, w1 = q * 128, (q + 1) * 128
                    for base in (0, TC):
                        nc.tensor.matmul(ps_ctx[:, base + w0:base + w1], vst,
                                         pt2[:, base + w0:base + w1],
                                         start=first, stop=True)
                        if w1 < TC:
                            nc.tensor.matmul(ps_ctx[:, base + w1:base + TC],
                                             vst, pt2[:, base + w1:base + TC],
                                             start=first, stop=False)
                    for ps_s, base in ((ps_s0, 0), (ps_s1, TC)):
                        nc.tensor.matmul(ps_s[:, w0:w1], ones_sb[:],
                                         pt2[:, base + w0:base + w1],
                                         start=first, stop=True)
                        if w1 < TC:
                            nc.tensor.matmul(ps_s[:, w1:TC], ones_sb[:],
                                             pt2[:, base + w1:base + TC],
                                             start=first, stop=False)

                for J in range(nj):
                    s2 = ps_pool.tile([128, 2 * TC], F32, tag="pair",
                                      name=f"pss_{I}_{hp}_{J}")
                    kst = kt_tiles[J // 4][:, (J % 4) * 128:(J % 4 + 1) * 128]
                    q = J - 4 * I
                    if q < 0:  # off-diagonal: full query range
                        nc.tensor.matmul(s2[:, 0:TC], kst, q_chunk[h0][:],
                                         start=True, stop=True)
                        nc.tensor.matmul(s2[:, TC:2 * TC], kst,
                                         q_chunk[h1][:],
                                         start=True, stop=True)
                        e0 = 0
                    else:
                        # diagonal: skip fully-masked i-subtiles, mask only
                        # the [128,128] triangle block (constant mask slice)
                        w0 = q * 128
                        nc.tensor.matmul(s2[:, w0:TC], kst,
                                         q_chunk[h0][:, w0:TC],
                                         start=True, stop=True)
                        nc.tensor.matmul(s2[:, TC + w0:2 * TC], kst,
                                         q_chunk[h1][:, w0:TC],
                                         start=True, stop=True)
                        nc.vector.tensor_add(
                            s2[:, w0:w0 + 128], s2[:, w0:w0 + 128],
                            mask_sb[:, 384:512]
                        )
                        nc.vector.tensor_add(
                            s2[:, TC + w0:TC + w0 + 128],
                            s2[:, TC + w0:TC + w0 + 128],
                            mask_sb[:, 384:512]
                        )
                        e0 = w0
                    pt2 = pt_pool.tile([128, 2 * TC], BF16, tag="pt",
                                       name=f"pt_{I}_{hp}_{J}")
                    nc.scalar.activation(pt2[:, e0:2 * TC], s2[:, e0:2 * TC],
                                         EXPF, scale=SCALE)
                    pts[J] = pt2
                    if J >= 1:
                        ctx_ones(J - 1)
                ctx_ones(nj - 1)

                # evacuate: ctx halves first (they gate the next chunk's
                # psum slots), then rowsum copies + fast [1,TC] reciprocals
                cx0 = cx_pool.tile([128, TC], BF16, tag="cx",
                                   name=f"cx_{I}_{h0}")
                nc.vector.tensor_copy(cx0[:], ps_ctx[:, 0:TC])
                ctx_un[h0] = cx0
                cx1 = cx_pool.tile([128, TC], BF16, tag="cx",
                                   name=f"cx_{I}_{h1}")
                nc.vector.tensor_copy(cx1[:], ps_ctx[:, TC:2 * TC])
                ctx_un[h1] = cx1
                for h, ps_s in ((h0, ps_s0), (h1, ps_s1)):
                    sl = sr_pool.tile([1, TC], F32, tag="sl", bufs=6,
                                      name=f"sl_{I}_{h}")
                    nc.scalar.copy(sl[:], ps_s[0:1, :])
                    nc.vector.reciprocal_approx_fast(sl[:], sl[:])
                    srh[h] = sl

            # per-head broadcast + scale — off the psum path
            for h in range(NH):
                rb = rb_pool.tile([128, TC], F32, tag="rb", name=f"rb_{I}_{h}")
                nc.gpsimd.partition_broadcast(rb[:], srh[h][:])
                cxt = ctx_un[h]
                nc.vector.tensor_mul(cxt[:], cxt[:], rb[:])
                cx_tiles[(h, I)] = cxt

        # ======= output stage: outT[e,t], resident wo stationary tiles =======
        for Et in range(NET):
            ps_o = [ps_pool.tile([128, 2 * TC], F32, tag="pair",
                                 name=f"pso_{Et}_{p}") for p in range(2)]
            for h in range(NH):
                wst = wo_sb[:, Et * DQ + h * HD:Et * DQ + (h + 1) * HD]
                for tc_ in range(NCH):
                    nc.tensor.matmul(
                        ps_o[tc_ // 2][:, (tc_ % 2) * TC:(tc_ % 2 + 1) * TC],
                        wst,
                        cx_tiles[(h, tc_)][:],
                        start=h == 0, stop=h == NH - 1,
                    )
            for p in range(2):
                ot = o_pool.tile([128, 2 * TC], BF16, tag="o",
                                 name=f"o_{Et}_{p}")
                if p == 0:
                    nc.vector.tensor_copy(ot[:], ps_o[p][:])
                else:
                    nc.scalar.copy(ot[:], ps_o[p][:])
                nc.sync.dma_start(
                    out[Et * 128:(Et + 1) * 128,
                        p * 2 * TC:(p + 1) * 2 * TC],
                    ot[:],
                )


# ---------------------------------------------------------------------------
# host side
# ---------------------------------------------------------------------------
_NC_CACHE = None


def _get_nc():
    global _NC_CACHE
    if _NC_CACHE is None:
        _NC_CACHE = build_nc()
    return _NC_CACHE


def _pmajor(w, kt, width):
    """[kt*128, width] -> partition-major [128, kt*width] bf16."""
    return np.ascontiguousarray(
        w.reshape(kt, 128, width).transpose(1, 0, 2).reshape(128, kt * width)
    )


def make_in_maps(x, Wq, Wk, Wv, Wo, cos, sin):
    x = np.asarray(x, dtype=np.float32)
    xT = np.ascontiguousarray(x.reshape(T, D).T.astype(NPBF16))
    cosT = np.ascontiguousarray(
        np.asarray(cos, np.float32)[:T].T.astype(NPBF16))
    sin_t = np.asarray(sin, np.float32)[:T]          # [T, 128]
    sinT = sin_t.T.copy()                            # [128, T]
    sinT[:64] *= -1.0                                # fold rotate-half sign
    sinT = np.ascontiguousarray(sinT.astype(NPBF16))

    # sliding additive causal mask: diagonal tile q reads cols
    # (3-q)*128 : (3-q)*128 + (q+1)*128 of big[r, cc] = 0 if cc >= 384 + r
    r = np.arange(128)[:, None]
    cc = np.arange(896)[None, :]
    m = np.where(cc >= 384 + r, 0.0, -1.0e30).astype(np.float32)
    ones = np.ones((128, 2), NPBF16)

    Wq = np.asarray(Wq, np.float32).astype(NPBF16)
    Wk = np.asarray(Wk, np.float32).astype(NPBF16)
    Wv = np.asarray(Wv, np.float32).astype(NPBF16)
    Wo = np.asarray(Wo, np.float32).astype(NPBF16)
    in_maps = []
    for g in range(NCORES):
        # wo rows [g*DQ:(g+1)*DQ] shuffled to [dh, (Et, h, e)] so Et-tiles are
        # resident stationary slices of one partition-major tensor
        w = Wo[g * DQ:(g + 1) * DQ, :]                          # [512, 4096]
        w4 = w.reshape(NH, HD, NET, 128).transpose(1, 2, 0, 3)  # [dh,Et,h,e]
        woP = np.ascontiguousarray(w4.reshape(128, NET * DQ))
        in_maps.append({
            "xT": xT,
            "wq": _pmajor(Wq[:, g * DQ:(g + 1) * DQ], KT, DQ),
            "wk": _pmajor(Wk[:, g * HD:(g + 1) * HD], KT, HD),
            "wv": _pmajor(Wv[:, g * HD:(g + 1) * HD], KT, HD),
            "wo": woP,
            "cosT": cosT,
            "sinT": sinT,
            "maskb": m,
            "ones": ones,
        })
    return in_maps


def kernel(x, Wq, Wk, Wv, Wo, cos, sin):
    nc = _get_nc()
    in_maps = make_in_maps(x, Wq, Wk, Wv, Wo, cos, sin)
    res = run_bass_kernel_spmd(nc, in_maps, core_ids=list(range(NCORES)))
    acc = np.zeros((D, T), np.float32)
    for c in range(NCORES):
        acc += res.results[c]["out"].astype(np.float32)
    return np.ascontiguousarray(acc.T).reshape(1, T, D)
